# revision 10
# baseline (speedup 1.0000x reference)
"""Trainium2 Bass kernel for AnemllQATLinear (fake-quant linear + LoRA + bias).

Math (per reference):
    scales = clip(scale_A @ scale_B, 1e-8)              # [OUT, IN], rank-4
    n      = w / scales
    q      = clip(round((n + 1) / step), 0, 15)         # step = 2/15
    w_q    = lut[q] * scales                            # lut affine: lut[q] = a + b*q
    y      = x @ w_q.T + bias + 2.0 * (x @ lora_A.T) @ lora_B.T

Strategy (8 NeuronCores, 4 row-groups x 2 col-groups):
    Each core gets x rows R=2048 and weight rows (out features) O=2048.
    - Fake-quant computed on-chip arithmetically (affine LUT; round via
      the +/-1.5*2^23 magic trick, which is round-half-even like jnp.round).
    - Quantized weight W_eff converted to bf16, transposed via the DMA
      x-bar into [in, out] layout (DRAM bounce), streamed per o-chunk.
    - x cast f32->bf16 during SWDGE DMA, x-bar transposed to xT resident.
    - Main matmul in bf16: psum[r, o] += xT.T @ weffT, with the LoRA
      term (rank-16) and bias (rank-1) accumulated into the same PSUM
      group as extra matmuls.
    - Output written f32, assembled (concat) on host.
"""

import os
import numpy as np

import concourse.bass as bass
import concourse.tile as tile
from concourse import bacc, mybir

F32 = mybir.dt.float32
F32R = mybir.dt.float32r
BF16 = mybir.dt.bfloat16
MAGIC = 12582912.0  # 1.5 * 2**23
LUT_SIZE = 16
STEP_INV = (LUT_SIZE - 1) / 2.0  # 7.5

# full problem shapes
B_FULL, S_FULL, IN_FULL, OUT_FULL = 4, 2048, 4096, 4096
RANK, LORA_R = 4, 16
R_GROUPS, O_GROUPS = 4, 2
N_CORES = 8


def build_nc(R, O, I, lut_a, lut_b, OC=256, IC=512, nonaffine_lut=None):
    """Build the single-core graph (SPMD-launched on all 8 cores).

    R: x rows per core; O: out features per core; I: contraction dim.
    OC: o-chunk for the main matmul (moving free dim). IC: i-chunk for quant.
    """
    KT = I // 128          # number of 128-wide i (contraction) tiles
    RT = R // 128          # r tiles
    ZC = min(512, R)       # z (lora) accumulation chunk of rows
    assert O % OC == 0 and OC % 128 == 0 and I % IC == 0 and IC % 128 == 0

    nc = bacc.Bacc(None, target_bir_lowering=False, debug=False)

    x_in = nc.declare_dram_parameter("x", [R, I], F32, isOutput=False)
    w_in = nc.declare_dram_parameter("w", [O, I], F32, isOutput=False)
    sAT_in = nc.declare_dram_parameter("sAT", [RANK, O], F32, isOutput=False)
    sB_in = nc.declare_dram_parameter("sB", [RANK, I], F32, isOutput=False)
    bias_in = nc.declare_dram_parameter("bias", [1, O], F32, isOutput=False)
    lAT_in = nc.declare_dram_parameter("lAT", [I, LORA_R], F32, isOutput=False)
    lBT_in = nc.declare_dram_parameter("lBT", [LORA_R, O], F32, isOutput=False)
    out_ext = nc.declare_dram_parameter("out", [R, O], F32, isOutput=True)

    with tile.TileContext(nc) as tc:
        with tc.tile_pool(name="dram", bufs=1, space="DRAM") as dram_pool, \
             tc.tile_pool(name="const", bufs=1) as const_pool, \
             tc.tile_pool(name="xside", bufs=2) as x_pool, \
             tc.tile_pool(name="xT", bufs=1) as xT_pool, \
             tc.tile_pool(name="wload", bufs=2) as w_pool, \
             tc.tile_pool(name="qs", bufs=2) as s_pool, \
             tc.tile_pool(name="qchain", bufs=3) as chain_pool, \
             tc.tile_pool(name="qout", bufs=2) as wq_pool, \
             tc.tile_pool(name="wTstage", bufs=2) as wT_pool, \
             tc.tile_pool(name="weffc", bufs=2) as weff_pool, \
             tc.tile_pool(name="ysb", bufs=2) as y_pool, \
             tc.tile_pool(name="smallio", bufs=2) as small_pool, \
             tc.tile_pool(name="ps_s", bufs=2, space="PSUM") as psum_s, \
             tc.tile_pool(name="ps_y", bufs=3, space="PSUM") as psum_y, \
             tc.tile_pool(name="ps_z", bufs=2, space="PSUM") as psum_z:

            weffT_dram = dram_pool.tile([128, KT, O], BF16)

            # ---- constants / small inputs ----
            lAT_sb = const_pool.tile([128, KT, LORA_R], BF16)
            nc.gpsimd.dma_start(
                out=lAT_sb[:],
                in_=lAT_in.rearrange("(kt p) r -> p kt r", p=128),
            )
            lBT2_sb = const_pool.tile([LORA_R, O], BF16)
            nc.gpsimd.dma_start(out=lBT2_sb[:], in_=lBT_in[:, :])  # cast f32->bf16
            bias_sb = const_pool.tile([1, O], BF16)
            nc.gpsimd.dma_start(out=bias_sb[:], in_=bias_in[:, :])  # cast f32->bf16
            ones_sb = const_pool.tile([1, 128], BF16)
            nc.vector.memset(ones_sb[:], 1.0)
            z_sb = const_pool.tile([LORA_R, R], BF16)

            # ---- x side: cast to bf16, x-bar transpose into resident xT ----
            xT_all = xT_pool.tile([128, KT, R], BF16)
            XH = 4  # quarter-tiles keep the staging pool small
            for rt in range(RT):
                for h in range(XH):
                    xbf = x_pool.tile([128, I // XH], BF16, tag="xbf")
                    nc.gpsimd.dma_start(
                        out=xbf[:],
                        in_=x_in[rt * 128:(rt + 1) * 128,
                                 h * (I // XH):(h + 1) * (I // XH)],
                    )
                    nc.sync.dma_start(
                        out=xT_all[:, h * (KT // XH):(h + 1) * (KT // XH),
                                   rt * 128:(rt + 1) * 128],
                        in_=xbf[:],
                        transpose=True,
                    )

            # ---- z = (x @ lora_A.T).T in bf16, scaled by 2 via lBT2 ----
            for zc in range(R // ZC):
                zp = psum_z.tile([LORA_R, ZC], F32, space="PSUM")
                for kt in range(KT):
                    nc.tensor.matmul(
                        zp[:],
                        lAT_sb[:, kt, :],
                        xT_all[:, kt, zc * ZC:(zc + 1) * ZC],
                        start=(kt == 0), stop=(kt == KT - 1),
                    )
                # fold the *2.0 LoRA scaling into the psum evacuation
                nc.vector.tensor_scalar(z_sb[:, zc * ZC:(zc + 1) * ZC], zp[:],
                                        2.0, None, op0=mybir.AluOpType.mult)

            # ---- weight quantization (per o-tile, per i-chunk) ----
            for ot in range(O // 128):
                sAT_f = small_pool.tile([RANK, 128], F32, tag="sATf")
                nc.gpsimd.dma_start(
                    out=sAT_f[:], in_=sAT_in[:, ot * 128:(ot + 1) * 128])
                sAT_t = small_pool.tile([RANK, 128], F32R, tag="sAT")
                nc.vector.tensor_copy(sAT_t[:], sAT_f[:])
                for ic in range(I // IC):
                    w_t = w_pool.tile([128, IC], F32, tag="w")
                    nc.gpsimd.dma_start(
                        out=w_t[:],
                        in_=w_in[ot * 128:(ot + 1) * 128, ic * IC:(ic + 1) * IC],
                    )
                    sB_f = small_pool.tile([RANK, IC], F32, tag="sBf")
                    nc.gpsimd.dma_start(
                        out=sB_f[:], in_=sB_in[:, ic * IC:(ic + 1) * IC])
                    sB_t = small_pool.tile([RANK, IC], F32R, tag="sB")
                    nc.vector.tensor_copy(sB_t[:], sB_f[:])
                    sp = psum_s.tile([128, IC], F32, space="PSUM")
                    nc.tensor.matmul(
                        sp[:],
                        sAT_t[:],
                        sB_t[:],
                        start=True, stop=True,
                    )
                    s_t = s_pool.tile([128, IC], F32, tag="s")
                    nc.vector.tensor_scalar(s_t[:], sp[:], 1e-8, None,
                                            op0=mybir.AluOpType.max)
                    r_t = chain_pool.tile([128, IC], F32, tag="chain")
                    scr = chain_pool.tile([128, IC], F32, tag="chain")
                    nc.vector.reciprocal_approx_accurate(r_t[:], s_t[:], scr[:])
                    n_t = chain_pool.tile([128, IC], F32, tag="chain")
                    nc.vector.tensor_tensor(n_t[:], w_t[:], r_t[:],
                                            op=mybir.AluOpType.mult)
                    v_t = chain_pool.tile([128, IC], F32, tag="chain")
                    nc.scalar.activation(v_t[:], n_t[:],
                                         mybir.ActivationFunctionType.Copy,
                                         bias=1.0 * STEP_INV, scale=STEP_INV)
                    t_t = chain_pool.tile([128, IC], F32, tag="chain")
                    nc.vector.tensor_scalar(t_t[:], v_t[:], MAGIC, MAGIC,
                                            op0=mybir.AluOpType.add,
                                            op1=mybir.AluOpType.subtract)
                    q_t = chain_pool.tile([128, IC], F32, tag="chain")
                    nc.vector.tensor_scalar(q_t[:], t_t[:], float(LUT_SIZE - 1), 0.0,
                                            op0=mybir.AluOpType.min,
                                            op1=mybir.AluOpType.max)
                    u_t = chain_pool.tile([128, IC], F32, tag="chain")
                    if nonaffine_lut is None:
                        nc.scalar.activation(u_t[:], q_t[:],
                                             mybir.ActivationFunctionType.Copy,
                                             bias=float(lut_a), scale=float(lut_b))
                    else:
                        # generic LUT: u = lut[0] + sum_k d_k * (q >= k - 0.5)
                        lut = nonaffine_lut
                        acc = chain_pool.tile([128, IC], F32, tag="chain")
                        nc.vector.tensor_scalar(acc[:], q_t[:], 0.0, float(lut[0]),
                                                op0=mybir.AluOpType.mult,
                                                op1=mybir.AluOpType.add)
                        for k in range(1, LUT_SIZE):
                            d_k = float(lut[k] - lut[k - 1])
                            ind = chain_pool.tile([128, IC], F32, tag="chain")
                            nc.vector.tensor_scalar(ind[:], q_t[:], k - 0.5, d_k,
                                                    op0=mybir.AluOpType.is_ge,
                                                    op1=mybir.AluOpType.mult)
                            acc2 = chain_pool.tile([128, IC], F32, tag="chain")
                            nc.vector.tensor_tensor(acc2[:], acc[:], ind[:],
                                                    op=mybir.AluOpType.add)
                            acc = acc2
                        u_t = acc
                    wq_t = wq_pool.tile([128, IC], BF16, tag="wq")
                    nc.vector.tensor_tensor(wq_t[:], u_t[:], s_t[:],
                                            op=mybir.AluOpType.mult)
                    # transpose to [i, o] layout and store to DRAM
                    wT_t = wT_pool.tile([128, IC // 128, 128], BF16, tag="wT")
                    nc.sync.dma_start(out=wT_t[:], in_=wq_t[:], transpose=True)
                    nc.gpsimd.dma_start(
                        out=weffT_dram[:, ic * (IC // 128):(ic + 1) * (IC // 128),
                                       ot * 128:(ot + 1) * 128],
                        in_=wT_t[:],
                    )

            # ---- main matmul: oc-outer, r-inner ----
            for oc in range(O // OC):
                weffT_c = weff_pool.tile([128, KT, OC], BF16, tag="weff")
                nc.gpsimd.dma_start(
                    out=weffT_c[:],
                    in_=weffT_dram[:, :, oc * OC:(oc + 1) * OC],
                )
                for rt in range(RT):
                    yp = psum_y.tile([128, OC], F32, space="PSUM")
                    for kt in range(KT):
                        nc.tensor.matmul(
                            yp[:],
                            xT_all[:, kt, rt * 128:(rt + 1) * 128],
                            weffT_c[:, kt, :],
                            start=(kt == 0), stop=False,
                        )
                    nc.tensor.matmul(
                        yp[:],
                        z_sb[:, rt * 128:(rt + 1) * 128],
                        lBT2_sb[:, oc * OC:(oc + 1) * OC],
                        start=False, stop=False,
                    )
                    nc.tensor.matmul(
                        yp[:],
                        ones_sb[:],
                        bias_sb[:, oc * OC:(oc + 1) * OC],
                        start=False, stop=True,
                    )
                    y_t = y_pool.tile([128, OC], F32, tag="y")
                    nc.scalar.copy(y_t[:], yp[:])
                    nc.gpsimd.dma_start(
                        out=out_ext[rt * 128:(rt + 1) * 128, oc * OC:(oc + 1) * OC],
                        in_=y_t[:],
                    )
    nc.compile()
    return nc


def _shard_inputs(x, weight, scale_A, scale_B, bias, lora_A, lora_B,
                  r_groups=R_GROUPS, o_groups=O_GROUPS):
    rows = x.shape[0]
    outs = weight.shape[0]
    Rs, Os = rows // r_groups, outs // o_groups
    lAT = np.ascontiguousarray(lora_A.T)
    in_maps = []
    for c in range(r_groups * o_groups):
        rg, og = divmod(c, o_groups)
        osl = slice(og * Os, (og + 1) * Os)
        in_maps.append({
            "x": np.ascontiguousarray(x[rg * Rs:(rg + 1) * Rs]),
            "w": np.ascontiguousarray(weight[osl]),
            "sAT": np.ascontiguousarray(scale_A[osl].T),
            "sB": np.ascontiguousarray(scale_B),
            "bias": np.ascontiguousarray(bias[osl][None, :]),
            "lAT": lAT,
            "lBT": np.ascontiguousarray(lora_B[osl].T),
        })
    return in_maps


_NC_CACHE = {}


def kernel(x, weight, scale_A, scale_B, bias, lora_A, lora_B, lut,
           _trace=False):
    from concourse.bass_utils import run_bass_kernel_spmd

    x = np.asarray(x, dtype=np.float32)
    weight = np.asarray(weight, dtype=np.float32)
    scale_A = np.asarray(scale_A, dtype=np.float32)
    scale_B = np.asarray(scale_B, dtype=np.float32)
    bias = np.asarray(bias, dtype=np.float32)
    lora_A = np.asarray(lora_A, dtype=np.float32)
    lora_B = np.asarray(lora_B, dtype=np.float32)
    lut = np.asarray(lut, dtype=np.float32)

    B, S, I = x.shape
    OUT = weight.shape[0]
    xf = x.reshape(B * S, I)
    R = (B * S) // R_GROUPS
    O = OUT // O_GROUPS

    d = np.diff(lut.astype(np.float64))
    affine = np.allclose(d, d[0], rtol=0, atol=1e-6 * max(1.0, np.abs(d[0])))
    lut_a = float(lut[0])
    lut_b = float(d.mean())
    nonaffine = None if affine else lut

    key = (R, O, I, lut_a, lut_b, affine)
    if key not in _NC_CACHE:
        _NC_CACHE[key] = build_nc(R, O, I, lut_a, lut_b,
                                  nonaffine_lut=nonaffine)
    nc = _NC_CACHE[key]

    in_maps = _shard_inputs(xf, weight, scale_A, scale_B, bias, lora_A, lora_B)
    res = run_bass_kernel_spmd(nc, in_maps, core_ids=list(range(N_CORES)),
                               trace=_trace)
    y = np.empty((B * S, OUT), np.float32)
    for c in range(N_CORES):
        rg, og = divmod(c, O_GROUPS)
        y[rg * R:(rg + 1) * R, og * O:(og + 1) * O] = \
            res.results[c]["out"].reshape(R, O)
    out = y.reshape(B, S, OUT)
    if _trace:
        return out, res
    return out


# revision 12
# speedup vs baseline: 1.1398x; 1.1398x over previous
"""Trainium2 Bass kernel for AnemllQATLinear (fake-quant linear + LoRA + bias).

Math (per reference):
    scales = clip(scale_A @ scale_B, 1e-8)              # [OUT, IN], rank-4
    n      = w / scales
    q      = clip(round((n + 1) / step), 0, 15)         # step = 2/15
    w_q    = lut[q] * scales                            # lut affine: lut[q] = a + b*q
    y      = x @ w_q.T + bias + 2.0 * (x @ lora_A.T) @ lora_B.T

Strategy (8 NeuronCores, 4 row-groups x 2 col-groups):
    Each core gets x rows R=2048 and weight rows (out features) O=2048.
    - Fake-quant computed on-chip arithmetically (affine LUT; round via
      the +/-1.5*2^23 magic trick, which is round-half-even like jnp.round).
    - Quantized weight W_eff converted to bf16, transposed via the DMA
      x-bar into [in, out] layout (DRAM bounce), streamed per o-chunk.
    - x cast f32->bf16 during SWDGE DMA, x-bar transposed to xT resident.
    - Main matmul in bf16: psum[r, o] += xT.T @ weffT, with the LoRA
      term (rank-16) and bias (rank-1) accumulated into the same PSUM
      group as extra matmuls.
    - Output written f32, assembled (concat) on host.
"""

import os
import numpy as np

import concourse.bass as bass
import concourse.tile as tile
from concourse import bacc, mybir

F32 = mybir.dt.float32
F32R = mybir.dt.float32r
BF16 = mybir.dt.bfloat16
MAGIC = 12582912.0  # 1.5 * 2**23
LUT_SIZE = 16
STEP_INV = (LUT_SIZE - 1) / 2.0  # 7.5

# full problem shapes
B_FULL, S_FULL, IN_FULL, OUT_FULL = 4, 2048, 4096, 4096
RANK, LORA_R = 4, 16
R_GROUPS, O_GROUPS = 4, 2
N_CORES = 8


def build_nc(R, O, I, lut_a, lut_b, OC=256, IC=512, nonaffine_lut=None):
    """Build the single-core graph (SPMD-launched on all 8 cores).

    R: x rows per core; O: out features per core; I: contraction dim.
    OC: o-chunk for the main matmul (moving free dim). IC: i-chunk for quant.
    """
    KT = I // 128          # number of 128-wide i (contraction) tiles
    RT = R // 128          # r tiles
    ZC = min(512, R)       # z (lora) accumulation chunk of rows
    assert O % OC == 0 and OC % 128 == 0 and I % IC == 0 and IC % 128 == 0

    nc = bacc.Bacc(None, target_bir_lowering=False, debug=False)

    x_in = nc.declare_dram_parameter("x", [R, I], F32, isOutput=False)
    w_in = nc.declare_dram_parameter("w", [O, I], F32, isOutput=False)
    sAT_in = nc.declare_dram_parameter("sAT", [RANK, O], F32, isOutput=False)
    sB_in = nc.declare_dram_parameter("sB", [RANK, I], F32, isOutput=False)
    bias_in = nc.declare_dram_parameter("bias", [1, O], F32, isOutput=False)
    lAT_in = nc.declare_dram_parameter("lAT", [I, LORA_R], F32, isOutput=False)
    lBT_in = nc.declare_dram_parameter("lBT", [LORA_R, O], F32, isOutput=False)
    out_ext = nc.declare_dram_parameter("out", [R, O], F32, isOutput=True)

    with tile.TileContext(nc) as tc:
        with              tc.tile_pool(name="const", bufs=1) as const_pool, \
             tc.tile_pool(name="xside", bufs=2) as x_pool, \
             tc.tile_pool(name="xT", bufs=1) as xT_pool, \
             tc.tile_pool(name="wload", bufs=2) as w_pool, \
             tc.tile_pool(name="qs", bufs=2) as s_pool, \
             tc.tile_pool(name="qchain", bufs=3) as chain_pool, \
             tc.tile_pool(name="qout", bufs=2) as wq_pool, \
             tc.tile_pool(name="weffc", bufs=2) as weff_pool, \
             tc.tile_pool(name="ysb", bufs=2) as y_pool, \
             tc.tile_pool(name="smallio", bufs=2) as small_pool, \
             tc.tile_pool(name="ps_s", bufs=2, space="PSUM") as psum_s, \
             tc.tile_pool(name="ps_y", bufs=3, space="PSUM") as psum_y, \
             tc.tile_pool(name="ps_z", bufs=2, space="PSUM") as psum_z:

            # ---- constants / small inputs ----
            lAT_sb = const_pool.tile([128, KT, LORA_R], BF16)
            nc.gpsimd.dma_start(
                out=lAT_sb[:],
                in_=lAT_in.rearrange("(kt p) r -> p kt r", p=128),
            )
            lBT2_sb = const_pool.tile([LORA_R, O], BF16)
            nc.gpsimd.dma_start(out=lBT2_sb[:], in_=lBT_in[:, :])  # cast f32->bf16
            bias_sb = const_pool.tile([1, O], BF16)
            nc.gpsimd.dma_start(out=bias_sb[:], in_=bias_in[:, :])  # cast f32->bf16
            ones_sb = const_pool.tile([1, 128], BF16)
            nc.vector.memset(ones_sb[:], 1.0)
            z_sb = const_pool.tile([LORA_R, R], BF16)

            # ---- x side: cast to bf16, x-bar transpose into resident xT ----
            xT_all = xT_pool.tile([128, KT, R], BF16)
            XH = 4  # quarter-tiles keep the staging pool small
            for rt in range(RT):
                for h in range(XH):
                    xbf = x_pool.tile([128, I // XH], BF16, tag="xbf")
                    nc.gpsimd.dma_start(
                        out=xbf[:],
                        in_=x_in[rt * 128:(rt + 1) * 128,
                                 h * (I // XH):(h + 1) * (I // XH)],
                    )
                    nc.sync.dma_start(
                        out=xT_all[:, h * (KT // XH):(h + 1) * (KT // XH),
                                   rt * 128:(rt + 1) * 128],
                        in_=xbf[:],
                        transpose=True,
                    )

            # ---- z = (x @ lora_A.T).T in bf16, scaled by 2 via lBT2 ----
            for zc in range(R // ZC):
                zp = psum_z.tile([LORA_R, ZC], F32, space="PSUM")
                for kt in range(KT):
                    nc.tensor.matmul(
                        zp[:],
                        lAT_sb[:, kt, :],
                        xT_all[:, kt, zc * ZC:(zc + 1) * ZC],
                        start=(kt == 0), stop=(kt == KT - 1),
                    )
                # fold the *2.0 LoRA scaling into the psum evacuation
                nc.vector.tensor_scalar(z_sb[:, zc * ZC:(zc + 1) * ZC], zp[:],
                                        2.0, None, op0=mybir.AluOpType.mult)

            # ---- main loop: per o-chunk, quantize the chunk then matmul it.
            # Trace order interleaves quant(oc) ahead of matmuls(oc) so the
            # scheduler overlaps quant(oc+1) on DVE/ACT with matmuls(oc) on PE.
            for oc in range(O // OC):
                weffT_c = weff_pool.tile([128, KT, OC], BF16, tag="weff")
                for ot_l in range(OC // 128):
                    ot = oc * (OC // 128) + ot_l
                    sAT_f = small_pool.tile([RANK, 128], F32, tag="sATf")
                    nc.scalar.dma_start(
                        out=sAT_f[:], in_=sAT_in[:, ot * 128:(ot + 1) * 128])
                    sAT_t = small_pool.tile([RANK, 128], F32R, tag="sAT")
                    nc.vector.tensor_copy(sAT_t[:], sAT_f[:])
                    for ic in range(I // IC):
                        w_t = w_pool.tile([128, IC], F32, tag="w")
                        nc.scalar.dma_start(
                            out=w_t[:],
                            in_=w_in[ot * 128:(ot + 1) * 128,
                                     ic * IC:(ic + 1) * IC],
                        )
                        sB_f = small_pool.tile([RANK, IC], F32, tag="sBf")
                        nc.scalar.dma_start(
                            out=sB_f[:], in_=sB_in[:, ic * IC:(ic + 1) * IC])
                        sB_t = small_pool.tile([RANK, IC], F32R, tag="sB")
                        nc.vector.tensor_copy(sB_t[:], sB_f[:])
                        sp = psum_s.tile([128, IC], F32, space="PSUM")
                        nc.tensor.matmul(
                            sp[:],
                            sAT_t[:],
                            sB_t[:],
                            start=True, stop=True,
                        )
                        s_t = s_pool.tile([128, IC], F32, tag="s")
                        nc.vector.tensor_scalar(s_t[:], sp[:], 1e-8, None,
                                                op0=mybir.AluOpType.max)
                        r_t = chain_pool.tile([128, IC], F32, tag="chain")
                        nc.vector.reciprocal_approx_fast(r_t[:], s_t[:])
                        n_t = chain_pool.tile([128, IC], F32, tag="chain")
                        nc.vector.tensor_tensor(n_t[:], w_t[:], r_t[:],
                                                op=mybir.AluOpType.mult)
                        v_t = chain_pool.tile([128, IC], F32, tag="chain")
                        nc.scalar.activation(v_t[:], n_t[:],
                                             mybir.ActivationFunctionType.Copy,
                                             bias=1.0 * STEP_INV, scale=STEP_INV)
                        t_t = chain_pool.tile([128, IC], F32, tag="chain")
                        nc.vector.tensor_scalar(t_t[:], v_t[:], MAGIC, MAGIC,
                                                op0=mybir.AluOpType.add,
                                                op1=mybir.AluOpType.subtract)
                        q_t = chain_pool.tile([128, IC], F32, tag="chain")
                        nc.vector.tensor_scalar(q_t[:], t_t[:],
                                                float(LUT_SIZE - 1), 0.0,
                                                op0=mybir.AluOpType.min,
                                                op1=mybir.AluOpType.max)
                        u_t = chain_pool.tile([128, IC], F32, tag="chain")
                        if nonaffine_lut is None:
                            nc.scalar.activation(u_t[:], q_t[:],
                                                 mybir.ActivationFunctionType.Copy,
                                                 bias=float(lut_a),
                                                 scale=float(lut_b))
                        else:
                            # generic LUT: u = lut[0] + sum_k d_k*(q >= k-0.5)
                            lut = nonaffine_lut
                            acc = chain_pool.tile([128, IC], F32, tag="chain")
                            nc.vector.tensor_scalar(acc[:], q_t[:], 0.0,
                                                    float(lut[0]),
                                                    op0=mybir.AluOpType.mult,
                                                    op1=mybir.AluOpType.add)
                            for k in range(1, LUT_SIZE):
                                d_k = float(lut[k] - lut[k - 1])
                                ind = chain_pool.tile([128, IC], F32, tag="chain")
                                nc.vector.tensor_scalar(ind[:], q_t[:], k - 0.5,
                                                        d_k,
                                                        op0=mybir.AluOpType.is_ge,
                                                        op1=mybir.AluOpType.mult)
                                acc2 = chain_pool.tile([128, IC], F32, tag="chain")
                                nc.vector.tensor_tensor(acc2[:], acc[:], ind[:],
                                                        op=mybir.AluOpType.add)
                                acc = acc2
                            u_t = acc
                        wq_t = wq_pool.tile([128, IC], BF16, tag="wq")
                        nc.vector.tensor_tensor(wq_t[:], u_t[:], s_t[:],
                                                op=mybir.AluOpType.mult)
                        # transpose straight into the weffT chunk
                        nc.sync.dma_start(
                            out=weffT_c[:, ic * (IC // 128):(ic + 1) * (IC // 128),
                                        ot_l * 128:(ot_l + 1) * 128],
                            in_=wq_t[:],
                            transpose=True,
                        )
                for rt in range(RT):
                    yp = psum_y.tile([128, OC], F32, space="PSUM")
                    for kt in range(KT):
                        nc.tensor.matmul(
                            yp[:],
                            xT_all[:, kt, rt * 128:(rt + 1) * 128],
                            weffT_c[:, kt, :],
                            start=(kt == 0), stop=False,
                        )
                    nc.tensor.matmul(
                        yp[:],
                        z_sb[:, rt * 128:(rt + 1) * 128],
                        lBT2_sb[:, oc * OC:(oc + 1) * OC],
                        start=False, stop=False,
                    )
                    nc.tensor.matmul(
                        yp[:],
                        ones_sb[:],
                        bias_sb[:, oc * OC:(oc + 1) * OC],
                        start=False, stop=True,
                    )
                    y_t = y_pool.tile([128, OC], F32, tag="y")
                    nc.scalar.copy(y_t[:], yp[:])
                    nc.scalar.dma_start(
                        out=out_ext[rt * 128:(rt + 1) * 128, oc * OC:(oc + 1) * OC],
                        in_=y_t[:],
                    )
    nc.compile()
    return nc


def _shard_inputs(x, weight, scale_A, scale_B, bias, lora_A, lora_B,
                  r_groups=R_GROUPS, o_groups=O_GROUPS):
    rows = x.shape[0]
    outs = weight.shape[0]
    Rs, Os = rows // r_groups, outs // o_groups
    lAT = np.ascontiguousarray(lora_A.T)
    in_maps = []
    for c in range(r_groups * o_groups):
        rg, og = divmod(c, o_groups)
        osl = slice(og * Os, (og + 1) * Os)
        in_maps.append({
            "x": np.ascontiguousarray(x[rg * Rs:(rg + 1) * Rs]),
            "w": np.ascontiguousarray(weight[osl]),
            "sAT": np.ascontiguousarray(scale_A[osl].T),
            "sB": np.ascontiguousarray(scale_B),
            "bias": np.ascontiguousarray(bias[osl][None, :]),
            "lAT": lAT,
            "lBT": np.ascontiguousarray(lora_B[osl].T),
        })
    return in_maps


_NC_CACHE = {}


def kernel(x, weight, scale_A, scale_B, bias, lora_A, lora_B, lut,
           _trace=False):
    from concourse.bass_utils import run_bass_kernel_spmd

    x = np.asarray(x, dtype=np.float32)
    weight = np.asarray(weight, dtype=np.float32)
    scale_A = np.asarray(scale_A, dtype=np.float32)
    scale_B = np.asarray(scale_B, dtype=np.float32)
    bias = np.asarray(bias, dtype=np.float32)
    lora_A = np.asarray(lora_A, dtype=np.float32)
    lora_B = np.asarray(lora_B, dtype=np.float32)
    lut = np.asarray(lut, dtype=np.float32)

    B, S, I = x.shape
    OUT = weight.shape[0]
    xf = x.reshape(B * S, I)
    R = (B * S) // R_GROUPS
    O = OUT // O_GROUPS

    d = np.diff(lut.astype(np.float64))
    affine = np.allclose(d, d[0], rtol=0, atol=1e-6 * max(1.0, np.abs(d[0])))
    lut_a = float(lut[0])
    lut_b = float(d.mean())
    nonaffine = None if affine else lut

    key = (R, O, I, lut_a, lut_b, affine)
    if key not in _NC_CACHE:
        _NC_CACHE[key] = build_nc(R, O, I, lut_a, lut_b,
                                  nonaffine_lut=nonaffine)
    nc = _NC_CACHE[key]

    in_maps = _shard_inputs(xf, weight, scale_A, scale_B, bias, lora_A, lora_B)
    res = run_bass_kernel_spmd(nc, in_maps, core_ids=list(range(N_CORES)),
                               trace=_trace)
    y = np.empty((B * S, OUT), np.float32)
    for c in range(N_CORES):
        rg, og = divmod(c, O_GROUPS)
        y[rg * R:(rg + 1) * R, og * O:(og + 1) * O] = \
            res.results[c]["out"].reshape(R, O)
    out = y.reshape(B, S, OUT)
    if _trace:
        return out, res
    return out


# revision 16
# speedup vs baseline: 1.2225x; 1.0726x over previous
"""Trainium2 Bass kernel for AnemllQATLinear (fake-quant linear + LoRA + bias).

Math (per reference):
    scales = clip(scale_A @ scale_B, 1e-8)              # [OUT, IN], rank-4
    n      = w / scales
    q      = clip(round((n + 1) / step), 0, 15)         # step = 2/15
    w_q    = lut[q] * scales                            # lut affine: lut[q] = a + b*q
    y      = x @ w_q.T + bias + 2.0 * (x @ lora_A.T) @ lora_B.T

Strategy (8 NeuronCores, 4 row-groups x 2 col-groups):
    Each core gets x rows R=2048 and weight rows (out features) O=2048.
    - Fake-quant computed on-chip arithmetically (affine LUT; round via
      the +/-1.5*2^23 magic trick, which is round-half-even like jnp.round).
    - Quantized weight W_eff converted to bf16, transposed via the DMA
      x-bar into [in, out] layout (DRAM bounce), streamed per o-chunk.
    - x cast f32->bf16 during SWDGE DMA, x-bar transposed to xT resident.
    - Main matmul in bf16: psum[r, o] += xT.T @ weffT, with the LoRA
      term (rank-16) and bias (rank-1) accumulated into the same PSUM
      group as extra matmuls.
    - Output written f32, assembled (concat) on host.
"""

import os
import numpy as np

import concourse.bass as bass
import concourse.tile as tile
from concourse import bacc, mybir

F32 = mybir.dt.float32
F32R = mybir.dt.float32r
BF16 = mybir.dt.bfloat16
MAGIC = 12582912.0  # 1.5 * 2**23
LUT_SIZE = 16
STEP_INV = (LUT_SIZE - 1) / 2.0  # 7.5

# full problem shapes
B_FULL, S_FULL, IN_FULL, OUT_FULL = 4, 2048, 4096, 4096
RANK, LORA_R = 4, 16
R_GROUPS, O_GROUPS = 4, 2
N_CORES = 8


def build_nc(R, O, I, lut_a, lut_b, OC=256, IC=512, nonaffine_lut=None):
    """Build the single-core graph (SPMD-launched on all 8 cores).

    R: x rows per core; O: out features per core; I: contraction dim.
    OC: o-chunk for the main matmul (moving free dim). IC: i-chunk for quant.
    """
    KT = I // 128          # number of 128-wide i (contraction) tiles
    RT = R // 128          # r tiles
    ZC = min(512, R)       # z (lora) accumulation chunk of rows
    assert O % OC == 0 and OC % 128 == 0 and I % IC == 0 and IC % 128 == 0

    nc = bacc.Bacc(None, target_bir_lowering=False, debug=False)

    x_in = nc.declare_dram_parameter("x", [R, I], F32, isOutput=False)
    w_in = nc.declare_dram_parameter("w", [O, I], F32, isOutput=False)
    sAT_in = nc.declare_dram_parameter("sAT", [RANK, O], F32, isOutput=False)
    sB_in = nc.declare_dram_parameter("sB", [RANK, I], F32, isOutput=False)
    bias_in = nc.declare_dram_parameter("bias", [1, O], F32, isOutput=False)
    lAT_in = nc.declare_dram_parameter("lAT", [I, LORA_R], F32, isOutput=False)
    lBT_in = nc.declare_dram_parameter("lBT", [LORA_R, O], F32, isOutput=False)
    out_ext = nc.declare_dram_parameter("out", [O, R], F32, isOutput=True)

    with tile.TileContext(nc) as tc:
        with              tc.tile_pool(name="const", bufs=1) as const_pool, \
             tc.tile_pool(name="xside", bufs=2) as x_pool, \
             tc.tile_pool(name="xT", bufs=1) as xT_pool, \
             tc.tile_pool(name="wload", bufs=2) as w_pool, \
             tc.tile_pool(name="qs", bufs=2) as s_pool, \
             tc.tile_pool(name="qchain", bufs=3) as chain_pool, \
             tc.tile_pool(name="qout", bufs=2) as wq_pool, \
             tc.tile_pool(name="weffc", bufs=3) as weff_pool, \
             tc.tile_pool(name="ysb", bufs=2) as y_pool, \
             tc.tile_pool(name="smallio", bufs=2) as small_pool, \
             tc.tile_pool(name="ps_s", bufs=2, space="PSUM") as psum_s, \
             tc.tile_pool(name="ps_y", bufs=1, space="PSUM") as psum_y, \
             tc.tile_pool(name="ps_z", bufs=2, space="PSUM") as psum_z:

            # ---- constants / small inputs ----
            lAT_sb = const_pool.tile([128, KT, LORA_R], BF16)
            nc.gpsimd.dma_start(
                out=lAT_sb[:],
                in_=lAT_in.rearrange("(kt p) r -> p kt r", p=128),
            )
            lBT2_sb = const_pool.tile([LORA_R, O], BF16)
            nc.gpsimd.dma_start(out=lBT2_sb[:], in_=lBT_in[:, :])  # cast f32->bf16
            bias_sb = const_pool.tile([1, O], BF16)
            nc.gpsimd.dma_start(out=bias_sb[:], in_=bias_in[:, :])  # cast f32->bf16
            ones_sb = const_pool.tile([1, 512], BF16)
            nc.vector.memset(ones_sb[:], 1.0)
            z_sb = const_pool.tile([LORA_R, R], BF16)

            # ---- x side: cast to bf16, x-bar transpose into resident xT ----
            xT_all = xT_pool.tile([128, KT, R], BF16)
            XH = 2  # half-tiles keep the staging pool small
            for rt in range(RT):
                for h in range(XH):
                    xbf = x_pool.tile([128, I // XH], BF16, tag="xbf")
                    nc.gpsimd.dma_start(
                        out=xbf[:],
                        in_=x_in[rt * 128:(rt + 1) * 128,
                                 h * (I // XH):(h + 1) * (I // XH)],
                    )
                    nc.sync.dma_start(
                        out=xT_all[:, h * (KT // XH):(h + 1) * (KT // XH),
                                   rt * 128:(rt + 1) * 128],
                        in_=xbf[:],
                        transpose=True,
                    )

            # ---- z = (x @ lora_A.T).T in bf16, scaled by 2 via lBT2 ----
            for zc in range(R // ZC):
                zp = psum_z.tile([LORA_R, ZC], F32, space="PSUM")
                for kt in range(KT):
                    nc.tensor.matmul(
                        zp[:],
                        lAT_sb[:, kt, :],
                        xT_all[:, kt, zc * ZC:(zc + 1) * ZC],
                        start=(kt == 0), stop=(kt == KT - 1),
                    )
                # fold the *2.0 LoRA scaling into the psum evacuation
                nc.vector.tensor_scalar(z_sb[:, zc * ZC:(zc + 1) * ZC], zp[:],
                                        2.0, None, op0=mybir.AluOpType.mult)

            # ---- main loop: per o-tile, quantize then matmul (yT = W_eff @ x.T).
            # Weight tiles are the stationary operand; x streams as the moving
            # operand in 512-row chunks, so each LDWEIGHTS feeds 4 matmuls.
            # Trace order lets the scheduler overlap quant(ot+1) on DVE/ACT
            # with matmuls(ot) on PE.
            RC = min(512, R)
            for ot in range(O // 128):
                weffT_c = weff_pool.tile([128, KT, 128], BF16, tag="weff")
                sAT_f = small_pool.tile([RANK, 128], F32, tag="sATf")
                nc.gpsimd.dma_start(
                    out=sAT_f[:], in_=sAT_in[:, ot * 128:(ot + 1) * 128])
                sAT_t = small_pool.tile([RANK, 128], F32R, tag="sAT")
                nc.vector.tensor_copy(sAT_t[:], sAT_f[:])
                for ic in range(I // IC):
                    w_t = w_pool.tile([128, IC], F32, tag="w")
                    nc.gpsimd.dma_start(
                        out=w_t[:],
                        in_=w_in[ot * 128:(ot + 1) * 128, ic * IC:(ic + 1) * IC],
                    )
                    sB_f = small_pool.tile([RANK, IC], F32, tag="sBf")
                    nc.gpsimd.dma_start(
                        out=sB_f[:], in_=sB_in[:, ic * IC:(ic + 1) * IC])
                    sB_t = small_pool.tile([RANK, IC], F32R, tag="sB")
                    nc.vector.tensor_copy(sB_t[:], sB_f[:])
                    sp = psum_s.tile([128, IC], F32, space="PSUM")
                    nc.tensor.matmul(sp[:], sAT_t[:], sB_t[:],
                                     start=True, stop=True)
                    s_t = s_pool.tile([128, IC], F32, tag="s")
                    nc.vector.tensor_scalar(s_t[:], sp[:], 1e-8, None,
                                            op0=mybir.AluOpType.max)
                    r_t = chain_pool.tile([128, IC], F32, tag="chain")
                    nc.vector.reciprocal_approx_fast(r_t[:], s_t[:])
                    n_t = chain_pool.tile([128, IC], F32, tag="chain")
                    nc.vector.tensor_tensor(n_t[:], w_t[:], r_t[:],
                                            op=mybir.AluOpType.mult)
                    v_t = chain_pool.tile([128, IC], F32, tag="chain")
                    nc.scalar.activation(v_t[:], n_t[:],
                                         mybir.ActivationFunctionType.Copy,
                                         bias=1.0 * STEP_INV, scale=STEP_INV)
                    t_t = chain_pool.tile([128, IC], F32, tag="chain")
                    nc.vector.tensor_scalar(t_t[:], v_t[:], MAGIC, MAGIC,
                                            op0=mybir.AluOpType.add,
                                            op1=mybir.AluOpType.subtract)
                    q_t = chain_pool.tile([128, IC], F32, tag="chain")
                    nc.vector.tensor_scalar(q_t[:], t_t[:],
                                            float(LUT_SIZE - 1), 0.0,
                                            op0=mybir.AluOpType.min,
                                            op1=mybir.AluOpType.max)
                    u_t = chain_pool.tile([128, IC], F32, tag="chain")
                    if nonaffine_lut is None:
                        nc.scalar.activation(u_t[:], q_t[:],
                                             mybir.ActivationFunctionType.Copy,
                                             bias=float(lut_a), scale=float(lut_b))
                    else:
                        # generic LUT: u = lut[0] + sum_k d_k*(q >= k-0.5)
                        lut = nonaffine_lut
                        acc = chain_pool.tile([128, IC], F32, tag="chain")
                        nc.vector.tensor_scalar(acc[:], q_t[:], 0.0, float(lut[0]),
                                                op0=mybir.AluOpType.mult,
                                                op1=mybir.AluOpType.add)
                        for k in range(1, LUT_SIZE):
                            d_k = float(lut[k] - lut[k - 1])
                            ind = chain_pool.tile([128, IC], F32, tag="chain")
                            nc.vector.tensor_scalar(ind[:], q_t[:], k - 0.5, d_k,
                                                    op0=mybir.AluOpType.is_ge,
                                                    op1=mybir.AluOpType.mult)
                            acc2 = chain_pool.tile([128, IC], F32, tag="chain")
                            nc.vector.tensor_tensor(acc2[:], acc[:], ind[:],
                                                    op=mybir.AluOpType.add)
                            acc = acc2
                        u_t = acc
                    wq_t = wq_pool.tile([128, IC], BF16, tag="wq")
                    nc.vector.tensor_tensor(wq_t[:], u_t[:], s_t[:],
                                            op=mybir.AluOpType.mult)
                    # transpose straight into the weffT tile
                    nc.sync.dma_start(
                        out=weffT_c[:, ic * (IC // 128):(ic + 1) * (IC // 128), :],
                        in_=wq_t[:],
                        transpose=True,
                    )
                # yT[o_tile, :] = W_eff[o_tile] @ x.T (+ lora + bias)
                yps = [psum_y.tile([128, RC], F32, space="PSUM",
                                   tag=f"yp{j}", name=f"yp{j}_{ot}")
                       for j in range(R // RC)]
                for kt in range(KT):
                    for j in range(R // RC):
                        nc.tensor.matmul(
                            yps[j][:],
                            weffT_c[:, kt, :],
                            xT_all[:, kt, j * RC:(j + 1) * RC],
                            start=(kt == 0), stop=False,
                        )
                for j in range(R // RC):
                    nc.tensor.matmul(
                        yps[j][:],
                        lBT2_sb[:, ot * 128:(ot + 1) * 128],
                        z_sb[:, j * RC:(j + 1) * RC],
                        start=False, stop=False,
                    )
                    nc.tensor.matmul(
                        yps[j][:],
                        bias_sb[:, ot * 128:(ot + 1) * 128],
                        ones_sb[:, :RC],
                        start=False, stop=True,
                    )
                    y_t = y_pool.tile([128, RC], F32, tag="y")
                    nc.scalar.copy(y_t[:], yps[j][:])
                    nc.gpsimd.dma_start(
                        out=out_ext[ot * 128:(ot + 1) * 128,
                                    j * RC:(j + 1) * RC],
                        in_=y_t[:],
                    )
    nc.compile()
    return nc


def _shard_inputs(x, weight, scale_A, scale_B, bias, lora_A, lora_B,
                  r_groups=R_GROUPS, o_groups=O_GROUPS):
    rows = x.shape[0]
    outs = weight.shape[0]
    Rs, Os = rows // r_groups, outs // o_groups
    lAT = np.ascontiguousarray(lora_A.T)
    in_maps = []
    for c in range(r_groups * o_groups):
        rg, og = divmod(c, o_groups)
        osl = slice(og * Os, (og + 1) * Os)
        in_maps.append({
            "x": np.ascontiguousarray(x[rg * Rs:(rg + 1) * Rs]),
            "w": np.ascontiguousarray(weight[osl]),
            "sAT": np.ascontiguousarray(scale_A[osl].T),
            "sB": np.ascontiguousarray(scale_B),
            "bias": np.ascontiguousarray(bias[osl][None, :]),
            "lAT": lAT,
            "lBT": np.ascontiguousarray(lora_B[osl].T),
        })
    return in_maps


_NC_CACHE = {}


def kernel(x, weight, scale_A, scale_B, bias, lora_A, lora_B, lut,
           _trace=False):
    from concourse.bass_utils import run_bass_kernel_spmd

    x = np.asarray(x, dtype=np.float32)
    weight = np.asarray(weight, dtype=np.float32)
    scale_A = np.asarray(scale_A, dtype=np.float32)
    scale_B = np.asarray(scale_B, dtype=np.float32)
    bias = np.asarray(bias, dtype=np.float32)
    lora_A = np.asarray(lora_A, dtype=np.float32)
    lora_B = np.asarray(lora_B, dtype=np.float32)
    lut = np.asarray(lut, dtype=np.float32)

    B, S, I = x.shape
    OUT = weight.shape[0]
    xf = x.reshape(B * S, I)
    R = (B * S) // R_GROUPS
    O = OUT // O_GROUPS

    d = np.diff(lut.astype(np.float64))
    affine = np.allclose(d, d[0], rtol=0, atol=1e-6 * max(1.0, np.abs(d[0])))
    lut_a = float(lut[0])
    lut_b = float(d.mean())
    nonaffine = None if affine else lut

    key = (R, O, I, lut_a, lut_b, affine)
    if key not in _NC_CACHE:
        _NC_CACHE[key] = build_nc(R, O, I, lut_a, lut_b,
                                  nonaffine_lut=nonaffine)
    nc = _NC_CACHE[key]

    in_maps = _shard_inputs(xf, weight, scale_A, scale_B, bias, lora_A, lora_B)
    res = run_bass_kernel_spmd(nc, in_maps, core_ids=list(range(N_CORES)),
                               trace=_trace)
    y = np.empty((B * S, OUT), np.float32)
    for c in range(N_CORES):
        rg, og = divmod(c, O_GROUPS)
        y[rg * R:(rg + 1) * R, og * O:(og + 1) * O] = \
            res.results[c]["out"].reshape(O, R).T
    out = y.reshape(B, S, OUT)
    if _trace:
        return out, res
    return out


# revision 19
# speedup vs baseline: 1.2249x; 1.0020x over previous
"""Trainium2 Bass kernel for AnemllQATLinear (fake-quant linear + LoRA + bias).

Math (per reference):
    scales = clip(scale_A @ scale_B, 1e-8)              # [OUT, IN], rank-4
    n      = w / scales
    q      = clip(round((n + 1) / step), 0, 15)         # step = 2/15
    w_q    = lut[q] * scales                            # lut affine: lut[q] = a + b*q
    y      = x @ w_q.T + bias + 2.0 * (x @ lora_A.T) @ lora_B.T

Strategy (8 NeuronCores, 4 row-groups x 2 col-groups):
    Each core gets x rows R=2048 and weight rows (out features) O=2048.
    - Fake-quant computed on-chip arithmetically (affine LUT; round via
      the +/-1.5*2^23 magic trick, which is round-half-even like jnp.round).
    - Quantized weight W_eff converted to bf16, transposed via the DMA
      x-bar into [in, out] layout (DRAM bounce), streamed per o-chunk.
    - x cast f32->bf16 during SWDGE DMA, x-bar transposed to xT resident.
    - Main matmul in bf16: psum[r, o] += xT.T @ weffT, with the LoRA
      term (rank-16) and bias (rank-1) accumulated into the same PSUM
      group as extra matmuls.
    - Output written f32, assembled (concat) on host.
"""

import os
import numpy as np

import concourse.bass as bass
import concourse.tile as tile
from concourse import bacc, mybir

F32 = mybir.dt.float32
F32R = mybir.dt.float32r
BF16 = mybir.dt.bfloat16
MAGIC = 12582912.0  # 1.5 * 2**23
LUT_SIZE = 16
STEP_INV = (LUT_SIZE - 1) / 2.0  # 7.5

# full problem shapes
B_FULL, S_FULL, IN_FULL, OUT_FULL = 4, 2048, 4096, 4096
RANK, LORA_R = 4, 16
R_GROUPS, O_GROUPS = 4, 2
N_CORES = 8


def build_nc(R, O, I, lut_a, lut_b, OC=256, IC=512, nonaffine_lut=None):
    """Build the single-core graph (SPMD-launched on all 8 cores).

    R: x rows per core; O: out features per core; I: contraction dim.
    OC: o-chunk for the main matmul (moving free dim). IC: i-chunk for quant.
    """
    KT = I // 128          # number of 128-wide i (contraction) tiles
    RT = R // 128          # r tiles
    ZC = min(512, R)       # z (lora) accumulation chunk of rows
    assert O % OC == 0 and OC % 128 == 0 and I % IC == 0 and IC % 128 == 0

    nc = bacc.Bacc(None, target_bir_lowering=False, debug=False)

    x_in = nc.declare_dram_parameter("x", [R, I], F32, isOutput=False)
    w_in = nc.declare_dram_parameter("w", [O, I], F32, isOutput=False)
    sAT_in = nc.declare_dram_parameter("sAT", [RANK, O], F32, isOutput=False)
    sB_in = nc.declare_dram_parameter("sB", [RANK, I], F32, isOutput=False)
    bias_in = nc.declare_dram_parameter("bias", [1, O], F32, isOutput=False)
    lAT_in = nc.declare_dram_parameter("lAT", [I, LORA_R], F32, isOutput=False)
    lBT_in = nc.declare_dram_parameter("lBT", [LORA_R, O], F32, isOutput=False)
    out_ext = nc.declare_dram_parameter("out", [O, R], F32, isOutput=True)

    with tile.TileContext(nc) as tc:
        with              tc.tile_pool(name="const", bufs=1) as const_pool, \
             tc.tile_pool(name="xside", bufs=2) as x_pool, \
             tc.tile_pool(name="xT", bufs=1) as xT_pool, \
             tc.tile_pool(name="wload", bufs=2) as w_pool, \
             tc.tile_pool(name="qs", bufs=2) as s_pool, \
             tc.tile_pool(name="qchain", bufs=4) as chain_pool, \
             tc.tile_pool(name="qout", bufs=3) as wq_pool, \
             tc.tile_pool(name="weffc", bufs=2) as weff_pool, \
             tc.tile_pool(name="ysb", bufs=2) as y_pool, \
             tc.tile_pool(name="smallio", bufs=2) as small_pool, \
             tc.tile_pool(name="ps_s", bufs=2, space="PSUM") as psum_s, \
             tc.tile_pool(name="ps_y", bufs=1, space="PSUM") as psum_y, \
             tc.tile_pool(name="ps_z", bufs=2, space="PSUM") as psum_z:

            # ---- constants / small inputs ----
            lAT_sb = const_pool.tile([128, KT, LORA_R], BF16)
            nc.gpsimd.dma_start(
                out=lAT_sb[:],
                in_=lAT_in.rearrange("(kt p) r -> p kt r", p=128),
            )
            lBT2_sb = const_pool.tile([LORA_R, O], BF16)
            nc.gpsimd.dma_start(out=lBT2_sb[:], in_=lBT_in[:, :])  # cast f32->bf16
            bias_sb = const_pool.tile([1, O], BF16)
            nc.gpsimd.dma_start(out=bias_sb[:], in_=bias_in[:, :])  # cast f32->bf16
            ones_sb = const_pool.tile([1, 512], BF16)
            nc.vector.memset(ones_sb[:], 1.0)
            sB_r = const_pool.tile([RANK, I], F32R)
            for sc in range(I // 512):
                sB_f = small_pool.tile([RANK, 512], F32, tag="sBf")
                nc.gpsimd.dma_start(
                    out=sB_f[:], in_=sB_in[:, sc * 512:(sc + 1) * 512])
                nc.vector.tensor_copy(sB_r[:, sc * 512:(sc + 1) * 512],
                                      sB_f[:])
            z_sb = const_pool.tile([LORA_R, R], BF16)

            # ---- x side: cast to bf16, x-bar transpose into resident xT ----
            xT_all = xT_pool.tile([128, KT, R], BF16)
            XH = 4  # quarter-tiles keep the staging pool small
            for rt in range(RT):
                for h in range(XH):
                    xbf = x_pool.tile([128, I // XH], BF16, tag="xbf")
                    nc.gpsimd.dma_start(
                        out=xbf[:],
                        in_=x_in[rt * 128:(rt + 1) * 128,
                                 h * (I // XH):(h + 1) * (I // XH)],
                    )
                    nc.sync.dma_start(
                        out=xT_all[:, h * (KT // XH):(h + 1) * (KT // XH),
                                   rt * 128:(rt + 1) * 128],
                        in_=xbf[:],
                        transpose=True,
                    )

            # ---- z = (x @ lora_A.T).T in bf16, scaled by 2 via lBT2 ----
            for zc in range(R // ZC):
                zp = psum_z.tile([LORA_R, ZC], F32, space="PSUM")
                for kt in range(KT):
                    nc.tensor.matmul(
                        zp[:],
                        lAT_sb[:, kt, :],
                        xT_all[:, kt, zc * ZC:(zc + 1) * ZC],
                        start=(kt == 0), stop=(kt == KT - 1),
                    )
                # fold the *2.0 LoRA scaling into the psum evacuation
                nc.vector.tensor_scalar(z_sb[:, zc * ZC:(zc + 1) * ZC], zp[:],
                                        2.0, None, op0=mybir.AluOpType.mult)

            # ---- main loop: per o-tile, quantize then matmul (yT = W_eff @ x.T).
            # Weight tiles are the stationary operand; x streams as the moving
            # operand in 512-row chunks, so each LDWEIGHTS feeds 4 matmuls.
            # Trace order lets the scheduler overlap quant(ot+1) on DVE/ACT
            # with matmuls(ot) on PE.
            RC = min(512, R)
            for ot in range(O // 128):
                weffT_c = weff_pool.tile([128, KT, 128], BF16, tag="weff")
                sAT_f = small_pool.tile([RANK, 128], F32, tag="sATf")
                nc.gpsimd.dma_start(
                    out=sAT_f[:], in_=sAT_in[:, ot * 128:(ot + 1) * 128])
                sAT_t = small_pool.tile([RANK, 128], F32R, tag="sAT")
                nc.vector.tensor_copy(sAT_t[:], sAT_f[:])
                for ic in range(I // IC):
                    w_t = w_pool.tile([128, IC], F32, tag="w")
                    nc.gpsimd.dma_start(
                        out=w_t[:],
                        in_=w_in[ot * 128:(ot + 1) * 128, ic * IC:(ic + 1) * IC],
                    )
                    sp = psum_s.tile([128, IC], F32, space="PSUM")
                    nc.tensor.matmul(sp[:], sAT_t[:],
                                     sB_r[:, ic * IC:(ic + 1) * IC],
                                     start=True, stop=True)
                    s_t = s_pool.tile([128, IC], F32, tag="s")
                    nc.vector.tensor_scalar(s_t[:], sp[:], 1e-8, None,
                                            op0=mybir.AluOpType.max)
                    r_t = chain_pool.tile([128, IC], F32, tag="chain")
                    nc.vector.reciprocal_approx_fast(r_t[:], s_t[:])
                    n_t = chain_pool.tile([128, IC], F32, tag="chain")
                    nc.vector.tensor_tensor(n_t[:], w_t[:], r_t[:],
                                            op=mybir.AluOpType.mult)
                    v_t = chain_pool.tile([128, IC], F32, tag="chain")
                    nc.vector.tensor_scalar(v_t[:], n_t[:], STEP_INV, -0.5,
                                            op0=mybir.AluOpType.mult,
                                            op1=mybir.AluOpType.add)
                    t_t = chain_pool.tile([128, IC], F32, tag="chain")
                    nc.vector.tensor_scalar(t_t[:], v_t[:], MAGIC + 8.0, MAGIC,
                                            op0=mybir.AluOpType.add,
                                            op1=mybir.AluOpType.subtract)
                    q_t = chain_pool.tile([128, IC], F32, tag="chain")
                    nc.vector.tensor_scalar(q_t[:], t_t[:],
                                            float(LUT_SIZE - 1), 0.0,
                                            op0=mybir.AluOpType.min,
                                            op1=mybir.AluOpType.max)
                    u_t = chain_pool.tile([128, IC], F32, tag="chain")
                    if nonaffine_lut is None:
                        nc.vector.tensor_scalar(u_t[:], q_t[:], float(lut_b),
                                                float(lut_a),
                                                op0=mybir.AluOpType.mult,
                                                op1=mybir.AluOpType.add)
                    else:
                        # generic LUT: u = lut[0] + sum_k d_k*(q >= k-0.5)
                        lut = nonaffine_lut
                        acc = chain_pool.tile([128, IC], F32, tag="chain")
                        nc.vector.tensor_scalar(acc[:], q_t[:], 0.0, float(lut[0]),
                                                op0=mybir.AluOpType.mult,
                                                op1=mybir.AluOpType.add)
                        for k in range(1, LUT_SIZE):
                            d_k = float(lut[k] - lut[k - 1])
                            ind = chain_pool.tile([128, IC], F32, tag="chain")
                            nc.vector.tensor_scalar(ind[:], q_t[:], k - 0.5, d_k,
                                                    op0=mybir.AluOpType.is_ge,
                                                    op1=mybir.AluOpType.mult)
                            acc2 = chain_pool.tile([128, IC], F32, tag="chain")
                            nc.vector.tensor_tensor(acc2[:], acc[:], ind[:],
                                                    op=mybir.AluOpType.add)
                            acc = acc2
                        u_t = acc
                    wq_t = wq_pool.tile([128, IC], BF16, tag="wq")
                    nc.vector.tensor_tensor(wq_t[:], u_t[:], s_t[:],
                                            op=mybir.AluOpType.mult)
                    # transpose straight into the weffT tile
                    nc.sync.dma_start(
                        out=weffT_c[:, ic * (IC // 128):(ic + 1) * (IC // 128), :],
                        in_=wq_t[:],
                        transpose=True,
                    )
                # yT[o_tile, :] = W_eff[o_tile] @ x.T (+ lora + bias)
                yps = [psum_y.tile([128, RC], F32, space="PSUM",
                                   tag=f"yp{j}", name=f"yp{j}_{ot}")
                       for j in range(R // RC)]
                for kt in range(KT):
                    for j in range(R // RC):
                        nc.tensor.matmul(
                            yps[j][:],
                            weffT_c[:, kt, :],
                            xT_all[:, kt, j * RC:(j + 1) * RC],
                            start=(kt == 0), stop=False,
                        )
                for j in range(R // RC):
                    nc.tensor.matmul(
                        yps[j][:],
                        lBT2_sb[:, ot * 128:(ot + 1) * 128],
                        z_sb[:, j * RC:(j + 1) * RC],
                        start=False, stop=False,
                    )
                    nc.tensor.matmul(
                        yps[j][:],
                        bias_sb[:, ot * 128:(ot + 1) * 128],
                        ones_sb[:, :RC],
                        start=False, stop=True,
                    )
                    y_t = y_pool.tile([128, RC], F32, tag="y")
                    nc.scalar.copy(y_t[:], yps[j][:])
                    nc.gpsimd.dma_start(
                        out=out_ext[ot * 128:(ot + 1) * 128,
                                    j * RC:(j + 1) * RC],
                        in_=y_t[:],
                    )
    nc.compile()
    return nc


def _shard_inputs(x, weight, scale_A, scale_B, bias, lora_A, lora_B,
                  r_groups=R_GROUPS, o_groups=O_GROUPS):
    rows = x.shape[0]
    outs = weight.shape[0]
    Rs, Os = rows // r_groups, outs // o_groups
    lAT = np.ascontiguousarray(lora_A.T)
    in_maps = []
    for c in range(r_groups * o_groups):
        rg, og = divmod(c, o_groups)
        osl = slice(og * Os, (og + 1) * Os)
        in_maps.append({
            "x": np.ascontiguousarray(x[rg * Rs:(rg + 1) * Rs]),
            "w": np.ascontiguousarray(weight[osl]),
            "sAT": np.ascontiguousarray(scale_A[osl].T),
            "sB": np.ascontiguousarray(scale_B),
            "bias": np.ascontiguousarray(bias[osl][None, :]),
            "lAT": lAT,
            "lBT": np.ascontiguousarray(lora_B[osl].T),
        })
    return in_maps


_NC_CACHE = {}


def kernel(x, weight, scale_A, scale_B, bias, lora_A, lora_B, lut,
           _trace=False):
    from concourse.bass_utils import run_bass_kernel_spmd

    x = np.asarray(x, dtype=np.float32)
    weight = np.asarray(weight, dtype=np.float32)
    scale_A = np.asarray(scale_A, dtype=np.float32)
    scale_B = np.asarray(scale_B, dtype=np.float32)
    bias = np.asarray(bias, dtype=np.float32)
    lora_A = np.asarray(lora_A, dtype=np.float32)
    lora_B = np.asarray(lora_B, dtype=np.float32)
    lut = np.asarray(lut, dtype=np.float32)

    B, S, I = x.shape
    OUT = weight.shape[0]
    xf = x.reshape(B * S, I)
    R = (B * S) // R_GROUPS
    O = OUT // O_GROUPS

    d = np.diff(lut.astype(np.float64))
    affine = np.allclose(d, d[0], rtol=0, atol=1e-6 * max(1.0, np.abs(d[0])))
    lut_a = float(lut[0])
    lut_b = float(d.mean())
    nonaffine = None if affine else lut

    key = (R, O, I, lut_a, lut_b, affine)
    if key not in _NC_CACHE:
        _NC_CACHE[key] = build_nc(R, O, I, lut_a, lut_b,
                                  nonaffine_lut=nonaffine)
    nc = _NC_CACHE[key]

    in_maps = _shard_inputs(xf, weight, scale_A, scale_B, bias, lora_A, lora_B)
    res = run_bass_kernel_spmd(nc, in_maps, core_ids=list(range(N_CORES)),
                               trace=_trace)
    y = np.empty((B * S, OUT), np.float32)
    for c in range(N_CORES):
        rg, og = divmod(c, O_GROUPS)
        y[rg * R:(rg + 1) * R, og * O:(og + 1) * O] = \
            res.results[c]["out"].reshape(O, R).T
    out = y.reshape(B, S, OUT)
    if _trace:
        return out, res
    return out


# revision 20
# speedup vs baseline: 1.2635x; 1.0315x over previous
"""Trainium2 Bass kernel for AnemllQATLinear (fake-quant linear + LoRA + bias).

Math (per reference):
    scales = clip(scale_A @ scale_B, 1e-8)              # [OUT, IN], rank-4
    n      = w / scales
    q      = clip(round((n + 1) / step), 0, 15)         # step = 2/15
    w_q    = lut[q] * scales                            # lut affine: lut[q] = a + b*q
    y      = x @ w_q.T + bias + 2.0 * (x @ lora_A.T) @ lora_B.T

Strategy (8 NeuronCores, 4 row-groups x 2 col-groups):
    Each core gets x rows R=2048 and weight rows (out features) O=2048.
    - Fake-quant computed on-chip arithmetically (affine LUT; round via
      the +/-1.5*2^23 magic trick, which is round-half-even like jnp.round).
    - Quantized weight W_eff converted to bf16, transposed via the DMA
      x-bar into [in, out] layout (DRAM bounce), streamed per o-chunk.
    - x cast f32->bf16 during SWDGE DMA, x-bar transposed to xT resident.
    - Main matmul in bf16: psum[r, o] += xT.T @ weffT, with the LoRA
      term (rank-16) and bias (rank-1) accumulated into the same PSUM
      group as extra matmuls.
    - Output written f32, assembled (concat) on host.
"""

import os
import numpy as np

import concourse.bass as bass
import concourse.tile as tile
from concourse import bacc, mybir

F32 = mybir.dt.float32
F32R = mybir.dt.float32r
BF16 = mybir.dt.bfloat16
MAGIC = 12582912.0  # 1.5 * 2**23
LUT_SIZE = 16
STEP_INV = (LUT_SIZE - 1) / 2.0  # 7.5

# full problem shapes
B_FULL, S_FULL, IN_FULL, OUT_FULL = 4, 2048, 4096, 4096
RANK, LORA_R = 4, 16
R_GROUPS, O_GROUPS = 4, 2
N_CORES = 8


def build_nc(R, O, I, lut_a, lut_b, OC=256, IC=512, nonaffine_lut=None):
    """Build the single-core graph (SPMD-launched on all 8 cores).

    R: x rows per core; O: out features per core; I: contraction dim.
    OC: o-chunk for the main matmul (moving free dim). IC: i-chunk for quant.
    """
    KT = I // 128          # number of 128-wide i (contraction) tiles
    RT = R // 128          # r tiles
    ZC = min(512, R)       # z (lora) accumulation chunk of rows
    assert O % OC == 0 and OC % 128 == 0 and I % IC == 0 and IC % 128 == 0

    nc = bacc.Bacc(None, target_bir_lowering=False, debug=False)

    x_in = nc.declare_dram_parameter("x", [R, I], F32, isOutput=False)
    w_in = nc.declare_dram_parameter("w", [O, I], F32, isOutput=False)
    sAT_in = nc.declare_dram_parameter("sAT", [RANK, O], F32, isOutput=False)
    sB_in = nc.declare_dram_parameter("sB", [RANK, I], F32, isOutput=False)
    bias_in = nc.declare_dram_parameter("bias", [1, O], F32, isOutput=False)
    lAT_in = nc.declare_dram_parameter("lAT", [I, LORA_R], F32, isOutput=False)
    lBT_in = nc.declare_dram_parameter("lBT", [LORA_R, O], F32, isOutput=False)
    out_ext = nc.declare_dram_parameter("out", [O, R], F32, isOutput=True)

    with tile.TileContext(nc) as tc:
        with              tc.tile_pool(name="const", bufs=1) as const_pool, \
             tc.tile_pool(name="xside", bufs=2) as x_pool, \
             tc.tile_pool(name="xT", bufs=1) as xT_pool, \
             tc.tile_pool(name="wload", bufs=2) as w_pool, \
             tc.tile_pool(name="qs", bufs=2) as s_pool, \
             tc.tile_pool(name="qchain", bufs=4) as chain_pool, \
             tc.tile_pool(name="qout", bufs=3) as wq_pool, \
             tc.tile_pool(name="weffc", bufs=2) as weff_pool, \
             tc.tile_pool(name="ysb", bufs=2) as y_pool, \
             tc.tile_pool(name="smallio", bufs=2) as small_pool, \
             tc.tile_pool(name="ps_s", bufs=2, space="PSUM") as psum_s, \
             tc.tile_pool(name="ps_y", bufs=1, space="PSUM") as psum_y, \
             tc.tile_pool(name="ps_z", bufs=2, space="PSUM") as psum_z:

            # ---- constants / small inputs ----
            lAT_sb = const_pool.tile([128, KT, LORA_R], BF16)
            nc.gpsimd.dma_start(
                out=lAT_sb[:],
                in_=lAT_in.rearrange("(kt p) r -> p kt r", p=128),
            )
            lBT2_sb = const_pool.tile([LORA_R, O], BF16)
            nc.gpsimd.dma_start(out=lBT2_sb[:], in_=lBT_in[:, :])  # cast f32->bf16
            bias_sb = const_pool.tile([1, O], BF16)
            nc.gpsimd.dma_start(out=bias_sb[:], in_=bias_in[:, :])  # cast f32->bf16
            ones_sb = const_pool.tile([1, 512], BF16)
            nc.vector.memset(ones_sb[:], 1.0)
            sB_r = const_pool.tile([RANK, I], F32R)
            for sc in range(I // 512):
                sB_f = small_pool.tile([RANK, 512], F32, tag="sBf")
                nc.gpsimd.dma_start(
                    out=sB_f[:], in_=sB_in[:, sc * 512:(sc + 1) * 512])
                nc.vector.tensor_copy(sB_r[:, sc * 512:(sc + 1) * 512],
                                      sB_f[:])
            z_sb = const_pool.tile([LORA_R, R], BF16)

            # ---- x side: cast to bf16, x-bar transpose into resident xT ----
            xT_all = xT_pool.tile([128, KT, R], BF16)
            XH = 4  # quarter-tiles keep the staging pool small
            for rt in range(RT):
                for h in range(XH):
                    xbf = x_pool.tile([128, I // XH], BF16, tag="xbf")
                    nc.gpsimd.dma_start(
                        out=xbf[:],
                        in_=x_in[rt * 128:(rt + 1) * 128,
                                 h * (I // XH):(h + 1) * (I // XH)],
                    )
                    nc.sync.dma_start(
                        out=xT_all[:, h * (KT // XH):(h + 1) * (KT // XH),
                                   rt * 128:(rt + 1) * 128],
                        in_=xbf[:],
                        transpose=True,
                    )

            # ---- z = (x @ lora_A.T).T in bf16, scaled by 2 via lBT2 ----
            for zc in range(R // ZC):
                zp = psum_z.tile([LORA_R, ZC], F32, space="PSUM")
                for kt in range(KT):
                    nc.tensor.matmul(
                        zp[:],
                        lAT_sb[:, kt, :],
                        xT_all[:, kt, zc * ZC:(zc + 1) * ZC],
                        start=(kt == 0), stop=(kt == KT - 1),
                    )
                # fold the *2.0 LoRA scaling into the psum evacuation
                nc.vector.tensor_scalar(z_sb[:, zc * ZC:(zc + 1) * ZC], zp[:],
                                        2.0, None, op0=mybir.AluOpType.mult)

            # ---- main loop: software-pipelined by one o-tile: quant(ot+1) is
            # traced BEFORE the matmuls of ot so every engine queue (PE's
            # scales-mm especially) runs the next tile's quant work while the
            # current column's matmuls stream.
            RC = min(512, R)

            def quant(ot):
                """Quantize weight o-tile `ot` into a fresh weffT tile."""
                weffT_c = weff_pool.tile([128, KT, 128], BF16, tag="weff",
                                         name=f"weff_{ot}")
                sAT_f = small_pool.tile([RANK, 128], F32, tag="sATf")
                nc.gpsimd.dma_start(
                    out=sAT_f[:], in_=sAT_in[:, ot * 128:(ot + 1) * 128])
                sAT_t = small_pool.tile([RANK, 128], F32R, tag="sAT")
                nc.vector.tensor_copy(sAT_t[:], sAT_f[:])
                for ic in range(I // IC):
                    w_t = w_pool.tile([128, IC], F32, tag="w")
                    nc.gpsimd.dma_start(
                        out=w_t[:],
                        in_=w_in[ot * 128:(ot + 1) * 128, ic * IC:(ic + 1) * IC],
                    )
                    sp = psum_s.tile([128, IC], F32, space="PSUM")
                    nc.tensor.matmul(sp[:], sAT_t[:],
                                     sB_r[:, ic * IC:(ic + 1) * IC],
                                     start=True, stop=True)
                    s_t = s_pool.tile([128, IC], F32, tag="s")
                    nc.vector.tensor_scalar(s_t[:], sp[:], 1e-8, None,
                                            op0=mybir.AluOpType.max)
                    r_t = chain_pool.tile([128, IC], F32, tag="chain")
                    nc.vector.reciprocal_approx_fast(r_t[:], s_t[:])
                    n_t = chain_pool.tile([128, IC], F32, tag="chain")
                    nc.vector.tensor_tensor(n_t[:], w_t[:], r_t[:],
                                            op=mybir.AluOpType.mult)
                    v_t = chain_pool.tile([128, IC], F32, tag="chain")
                    nc.vector.tensor_scalar(v_t[:], n_t[:], STEP_INV, -0.5,
                                            op0=mybir.AluOpType.mult,
                                            op1=mybir.AluOpType.add)
                    t_t = chain_pool.tile([128, IC], F32, tag="chain")
                    nc.vector.tensor_scalar(t_t[:], v_t[:], MAGIC + 8.0, MAGIC,
                                            op0=mybir.AluOpType.add,
                                            op1=mybir.AluOpType.subtract)
                    q_t = chain_pool.tile([128, IC], F32, tag="chain")
                    nc.vector.tensor_scalar(q_t[:], t_t[:],
                                            float(LUT_SIZE - 1), 0.0,
                                            op0=mybir.AluOpType.min,
                                            op1=mybir.AluOpType.max)
                    u_t = chain_pool.tile([128, IC], F32, tag="chain")
                    if nonaffine_lut is None:
                        nc.vector.tensor_scalar(u_t[:], q_t[:], float(lut_b),
                                                float(lut_a),
                                                op0=mybir.AluOpType.mult,
                                                op1=mybir.AluOpType.add)
                    else:
                        # generic LUT: u = lut[0] + sum_k d_k*(q >= k-0.5)
                        lut = nonaffine_lut
                        acc = chain_pool.tile([128, IC], F32, tag="chain")
                        nc.vector.tensor_scalar(acc[:], q_t[:], 0.0, float(lut[0]),
                                                op0=mybir.AluOpType.mult,
                                                op1=mybir.AluOpType.add)
                        for k in range(1, LUT_SIZE):
                            d_k = float(lut[k] - lut[k - 1])
                            ind = chain_pool.tile([128, IC], F32, tag="chain")
                            nc.vector.tensor_scalar(ind[:], q_t[:], k - 0.5, d_k,
                                                    op0=mybir.AluOpType.is_ge,
                                                    op1=mybir.AluOpType.mult)
                            acc2 = chain_pool.tile([128, IC], F32, tag="chain")
                            nc.vector.tensor_tensor(acc2[:], acc[:], ind[:],
                                                    op=mybir.AluOpType.add)
                            acc = acc2
                        u_t = acc
                    wq_t = wq_pool.tile([128, IC], BF16, tag="wq")
                    nc.vector.tensor_tensor(wq_t[:], u_t[:], s_t[:],
                                            op=mybir.AluOpType.mult)
                    # transpose straight into the weffT tile
                    nc.sync.dma_start(
                        out=weffT_c[:, ic * (IC // 128):(ic + 1) * (IC // 128), :],
                        in_=wq_t[:],
                        transpose=True,
                    )
                return weffT_c

            def column(ot, weffT_c):
                """yT[o_tile, :] = W_eff[o_tile] @ x.T (+ lora + bias)."""
                yps = [psum_y.tile([128, RC], F32, space="PSUM",
                                   tag=f"yp{j}", name=f"yp{j}_{ot}")
                       for j in range(R // RC)]
                for kt in range(KT):
                    for j in range(R // RC):
                        nc.tensor.matmul(
                            yps[j][:],
                            weffT_c[:, kt, :],
                            xT_all[:, kt, j * RC:(j + 1) * RC],
                            start=(kt == 0), stop=False,
                        )
                for j in range(R // RC):
                    nc.tensor.matmul(
                        yps[j][:],
                        lBT2_sb[:, ot * 128:(ot + 1) * 128],
                        z_sb[:, j * RC:(j + 1) * RC],
                        start=False, stop=False,
                    )
                    nc.tensor.matmul(
                        yps[j][:],
                        bias_sb[:, ot * 128:(ot + 1) * 128],
                        ones_sb[:, :RC],
                        start=False, stop=True,
                    )
                    y_t = y_pool.tile([128, RC], F32, tag="y")
                    nc.scalar.copy(y_t[:], yps[j][:])
                    nc.gpsimd.dma_start(
                        out=out_ext[ot * 128:(ot + 1) * 128,
                                    j * RC:(j + 1) * RC],
                        in_=y_t[:],
                    )

            NT = O // 128
            pending = quant(0)
            for ot in range(NT):
                nxt = quant(ot + 1) if ot + 1 < NT else None
                column(ot, pending)
                pending = nxt
    nc.compile()
    return nc


def _shard_inputs(x, weight, scale_A, scale_B, bias, lora_A, lora_B,
                  r_groups=R_GROUPS, o_groups=O_GROUPS):
    rows = x.shape[0]
    outs = weight.shape[0]
    Rs, Os = rows // r_groups, outs // o_groups
    lAT = np.ascontiguousarray(lora_A.T)
    in_maps = []
    for c in range(r_groups * o_groups):
        rg, og = divmod(c, o_groups)
        osl = slice(og * Os, (og + 1) * Os)
        in_maps.append({
            "x": np.ascontiguousarray(x[rg * Rs:(rg + 1) * Rs]),
            "w": np.ascontiguousarray(weight[osl]),
            "sAT": np.ascontiguousarray(scale_A[osl].T),
            "sB": np.ascontiguousarray(scale_B),
            "bias": np.ascontiguousarray(bias[osl][None, :]),
            "lAT": lAT,
            "lBT": np.ascontiguousarray(lora_B[osl].T),
        })
    return in_maps


_NC_CACHE = {}


def kernel(x, weight, scale_A, scale_B, bias, lora_A, lora_B, lut,
           _trace=False):
    from concourse.bass_utils import run_bass_kernel_spmd

    x = np.asarray(x, dtype=np.float32)
    weight = np.asarray(weight, dtype=np.float32)
    scale_A = np.asarray(scale_A, dtype=np.float32)
    scale_B = np.asarray(scale_B, dtype=np.float32)
    bias = np.asarray(bias, dtype=np.float32)
    lora_A = np.asarray(lora_A, dtype=np.float32)
    lora_B = np.asarray(lora_B, dtype=np.float32)
    lut = np.asarray(lut, dtype=np.float32)

    B, S, I = x.shape
    OUT = weight.shape[0]
    xf = x.reshape(B * S, I)
    R = (B * S) // R_GROUPS
    O = OUT // O_GROUPS

    d = np.diff(lut.astype(np.float64))
    affine = np.allclose(d, d[0], rtol=0, atol=1e-6 * max(1.0, np.abs(d[0])))
    lut_a = float(lut[0])
    lut_b = float(d.mean())
    nonaffine = None if affine else lut

    key = (R, O, I, lut_a, lut_b, affine)
    if key not in _NC_CACHE:
        _NC_CACHE[key] = build_nc(R, O, I, lut_a, lut_b,
                                  nonaffine_lut=nonaffine)
    nc = _NC_CACHE[key]

    in_maps = _shard_inputs(xf, weight, scale_A, scale_B, bias, lora_A, lora_B)
    res = run_bass_kernel_spmd(nc, in_maps, core_ids=list(range(N_CORES)),
                               trace=_trace)
    y = np.empty((B * S, OUT), np.float32)
    for c in range(N_CORES):
        rg, og = divmod(c, O_GROUPS)
        y[rg * R:(rg + 1) * R, og * O:(og + 1) * O] = \
            res.results[c]["out"].reshape(O, R).T
    out = y.reshape(B, S, OUT)
    if _trace:
        return out, res
    return out


# revision 23
# speedup vs baseline: 1.4328x; 1.1340x over previous
"""Trainium2 Bass kernel for AnemllQATLinear (fake-quant linear + LoRA + bias).

Math (per reference):
    scales = clip(scale_A @ scale_B, 1e-8)              # [OUT, IN], rank-4
    n      = w / scales
    q      = clip(round((n + 1) / step), 0, 15)         # step = 2/15
    w_q    = lut[q] * scales                            # lut affine: lut[q] = a + b*q
    y      = x @ w_q.T + bias + 2.0 * (x @ lora_A.T) @ lora_B.T

Strategy (8 NeuronCores, 4 row-groups x 2 col-groups):
    Each core gets x rows R=2048 and weight rows (out features) O=2048.
    - Fake-quant computed on-chip arithmetically (affine LUT; round via
      the +/-1.5*2^23 magic trick, which is round-half-even like jnp.round).
    - Quantized weight W_eff converted to bf16, transposed via the DMA
      x-bar into [in, out] layout (DRAM bounce), streamed per o-chunk.
    - x cast f32->bf16 during SWDGE DMA, x-bar transposed to xT resident.
    - Main matmul in bf16: psum[r, o] += xT.T @ weffT, with the LoRA
      term (rank-16) and bias (rank-1) accumulated into the same PSUM
      group as extra matmuls.
    - Output written f32, assembled (concat) on host.
"""

import os
import numpy as np

import concourse.bass as bass
import concourse.tile as tile
from concourse import bacc, mybir

F32 = mybir.dt.float32
F32R = mybir.dt.float32r
BF16 = mybir.dt.bfloat16
MAGIC = 12582912.0  # 1.5 * 2**23
LUT_SIZE = 16
STEP_INV = (LUT_SIZE - 1) / 2.0  # 7.5

# full problem shapes
B_FULL, S_FULL, IN_FULL, OUT_FULL = 4, 2048, 4096, 4096
RANK, LORA_R = 4, 16
R_GROUPS, O_GROUPS = 4, 2
N_CORES = 8


def build_nc(R, O, I, lut_a, lut_b, OC=256, IC=512, nonaffine_lut=None):
    """Build the single-core graph (SPMD-launched on all 8 cores).

    R: x rows per core; O: out features per core; I: contraction dim.
    OC: o-chunk for the main matmul (moving free dim). IC: i-chunk for quant.
    """
    KT = I // 128          # number of 128-wide i (contraction) tiles
    RT = R // 128          # r tiles
    ZC = min(512, R)       # z (lora) accumulation chunk of rows
    assert O % OC == 0 and OC % 128 == 0 and I % IC == 0 and IC % 128 == 0

    nc = bacc.Bacc(None, target_bir_lowering=False, debug=False)

    x_in = nc.declare_dram_parameter("x", [R, I], F32, isOutput=False)
    w_in = nc.declare_dram_parameter("w", [O, I], F32, isOutput=False)
    sAT_in = nc.declare_dram_parameter("sAT", [RANK, O], F32, isOutput=False)
    sB_in = nc.declare_dram_parameter("sB", [RANK, I], F32, isOutput=False)
    bias_in = nc.declare_dram_parameter("bias", [1, O], F32, isOutput=False)
    lAT_in = nc.declare_dram_parameter("lAT", [I, LORA_R], F32, isOutput=False)
    lBT_in = nc.declare_dram_parameter("lBT", [LORA_R, O], F32, isOutput=False)
    out_ext = nc.declare_dram_parameter("out", [O, R], F32, isOutput=True)

    with tile.TileContext(nc) as tc:
        with              tc.tile_pool(name="const", bufs=1) as const_pool, \
             tc.tile_pool(name="xside", bufs=2) as x_pool, \
             tc.tile_pool(name="xT", bufs=1) as xT_pool, \
             tc.tile_pool(name="wload", bufs=2) as w_pool, \
             tc.tile_pool(name="qs", bufs=2) as s_pool, \
             tc.tile_pool(name="qchain", bufs=4) as chain_pool, \
             tc.tile_pool(name="qout", bufs=2) as wq_pool, \
             tc.tile_pool(name="weffc", bufs=3) as weff_pool, \
             tc.tile_pool(name="ysb", bufs=2) as y_pool, \
             tc.tile_pool(name="smallio", bufs=2) as small_pool, \
             tc.tile_pool(name="ps_s", bufs=2, space="PSUM") as psum_s, \
             tc.tile_pool(name="ps_y", bufs=1, space="PSUM") as psum_y, \
             tc.tile_pool(name="ps_z", bufs=2, space="PSUM") as psum_z:

            # ---- constants / small inputs ----
            lAT_sb = const_pool.tile([128, KT, LORA_R], BF16)
            nc.gpsimd.dma_start(
                out=lAT_sb[:],
                in_=lAT_in.rearrange("(kt p) r -> p kt r", p=128),
            )
            lBT2_sb = const_pool.tile([LORA_R, O], BF16)
            nc.gpsimd.dma_start(out=lBT2_sb[:], in_=lBT_in[:, :])  # cast f32->bf16
            bias_cols = const_pool.tile([128, O // 128], F32)
            nc.gpsimd.dma_start(
                out=bias_cols[:],
                in_=bias_in.rearrange("1 (ot p) -> p ot", p=128))
            sB_r = const_pool.tile([RANK, I], F32R)
            for sc in range(I // 256):
                sB_f = small_pool.tile([RANK, 256], F32, tag="sBf")
                nc.gpsimd.dma_start(
                    out=sB_f[:], in_=sB_in[:, sc * 256:(sc + 1) * 256])
                nc.vector.tensor_copy(sB_r[:, sc * 256:(sc + 1) * 256],
                                      sB_f[:])
            z_sb = const_pool.tile([LORA_R, R], BF16)

            # ---- x side: cast to bf16, x-bar transpose into per-chunk xT
            # tiles (finer tiles -> finer scheduler dependencies), with the
            # LoRA z matmuls interleaved per chunk.
            RC = min(512, R)
            NCH = R // RC
            xT_chunks = [xT_pool.tile([128, KT, RC], BF16, name=f"xT_{j}")
                         for j in range(NCH)]
            XH = 4  # quarter-tiles keep the staging pool small

            def x_side_and_z():
                for j in range(NCH):
                    for rt in range(j * (RC // 128), (j + 1) * (RC // 128)):
                        for h in range(XH):
                            xbf = x_pool.tile([128, I // XH], BF16, tag="xbf")
                            nc.gpsimd.dma_start(
                                out=xbf[:],
                                in_=x_in[rt * 128:(rt + 1) * 128,
                                         h * (I // XH):(h + 1) * (I // XH)],
                            )
                            lo = (rt % (RC // 128)) * 128
                            nc.sync.dma_start(
                                out=xT_chunks[j][:,
                                                 h * (KT // XH):(h + 1) * (KT // XH),
                                                 lo:lo + 128],
                                in_=xbf[:],
                                transpose=True,
                            )
                    # z for this chunk
                    zp = psum_z.tile([LORA_R, RC], F32, space="PSUM")
                    for kt in range(KT):
                        nc.tensor.matmul(
                            zp[:],
                            lAT_sb[:, kt, :],
                            xT_chunks[j][:, kt, :],
                            start=(kt == 0), stop=(kt == KT - 1),
                        )
                    # fold the *2.0 LoRA scaling into the psum evacuation
                    nc.vector.tensor_scalar(z_sb[:, j * RC:(j + 1) * RC], zp[:],
                                            2.0, None, op0=mybir.AluOpType.mult)

            # ---- main loop: software-pipelined by one o-tile: quant(ot+1) is
            # traced BEFORE the matmuls of ot so every engine queue (PE's
            # scales-mm especially) runs the next tile's quant work while the
            # current column's matmuls stream.

            def quant(ot):
                """Quantize weight o-tile `ot` into a fresh weffT tile."""
                weffT_c = weff_pool.tile([128, KT, 128], BF16, tag="weff",
                                         name=f"weff_{ot}")
                sAT_f = small_pool.tile([RANK, 128], F32, tag="sATf")
                nc.scalar.dma_start(
                    out=sAT_f[:], in_=sAT_in[:, ot * 128:(ot + 1) * 128])
                sAT_t = small_pool.tile([RANK, 128], F32R, tag="sAT")
                nc.vector.tensor_copy(sAT_t[:], sAT_f[:])
                for ic in range(I // IC):
                    w_t = w_pool.tile([128, IC], F32, tag="w")
                    nc.scalar.dma_start(
                        out=w_t[:],
                        in_=w_in[ot * 128:(ot + 1) * 128, ic * IC:(ic + 1) * IC],
                    )
                    sp = psum_s.tile([128, IC], F32, space="PSUM")
                    nc.tensor.matmul(sp[:], sAT_t[:],
                                     sB_r[:, ic * IC:(ic + 1) * IC],
                                     start=True, stop=True)
                    s_t = s_pool.tile([128, IC], F32, tag="s")
                    nc.vector.tensor_scalar(s_t[:], sp[:], 1e-8, None,
                                            op0=mybir.AluOpType.max)
                    r_t = chain_pool.tile([128, IC], F32, tag="chain")
                    nc.vector.reciprocal_approx_fast(r_t[:], s_t[:])
                    n_t = chain_pool.tile([128, IC], F32, tag="chain")
                    nc.vector.tensor_tensor(n_t[:], w_t[:], r_t[:],
                                            op=mybir.AluOpType.mult)
                    v_t = chain_pool.tile([128, IC], F32, tag="chain")
                    nc.vector.tensor_scalar(v_t[:], n_t[:], STEP_INV, -0.5,
                                            op0=mybir.AluOpType.mult,
                                            op1=mybir.AluOpType.add)
                    t_t = chain_pool.tile([128, IC], F32, tag="chain")
                    nc.vector.tensor_scalar(t_t[:], v_t[:], MAGIC + 8.0, MAGIC,
                                            op0=mybir.AluOpType.add,
                                            op1=mybir.AluOpType.subtract)
                    q_t = chain_pool.tile([128, IC], F32, tag="chain")
                    nc.vector.tensor_scalar(q_t[:], t_t[:],
                                            float(LUT_SIZE - 1), 0.0,
                                            op0=mybir.AluOpType.min,
                                            op1=mybir.AluOpType.max)
                    u_t = chain_pool.tile([128, IC], F32, tag="chain")
                    if nonaffine_lut is None:
                        nc.vector.tensor_scalar(u_t[:], q_t[:], float(lut_b),
                                                float(lut_a),
                                                op0=mybir.AluOpType.mult,
                                                op1=mybir.AluOpType.add)
                    else:
                        # generic LUT: u = lut[0] + sum_k d_k*(q >= k-0.5)
                        lut = nonaffine_lut
                        acc = chain_pool.tile([128, IC], F32, tag="chain")
                        nc.vector.tensor_scalar(acc[:], q_t[:], 0.0, float(lut[0]),
                                                op0=mybir.AluOpType.mult,
                                                op1=mybir.AluOpType.add)
                        for k in range(1, LUT_SIZE):
                            d_k = float(lut[k] - lut[k - 1])
                            ind = chain_pool.tile([128, IC], F32, tag="chain")
                            nc.vector.tensor_scalar(ind[:], q_t[:], k - 0.5, d_k,
                                                    op0=mybir.AluOpType.is_ge,
                                                    op1=mybir.AluOpType.mult)
                            acc2 = chain_pool.tile([128, IC], F32, tag="chain")
                            nc.vector.tensor_tensor(acc2[:], acc[:], ind[:],
                                                    op=mybir.AluOpType.add)
                            acc = acc2
                        u_t = acc
                    wq_t = wq_pool.tile([128, IC], BF16, tag="wq")
                    nc.vector.tensor_tensor(wq_t[:], u_t[:], s_t[:],
                                            op=mybir.AluOpType.mult)
                    # transpose straight into the weffT tile
                    nc.sync.dma_start(
                        out=weffT_c[:, ic * (IC // 128):(ic + 1) * (IC // 128), :],
                        in_=wq_t[:],
                        transpose=True,
                    )
                return weffT_c

            def column(ot, weffT_c):
                """yT[o_tile, :] = W_eff[o_tile] @ x.T + lora; bias folds
                into the ACT psum evacuation as a per-partition offset."""
                yps = [psum_y.tile([128, RC], F32, space="PSUM",
                                   tag=f"yp{j}", name=f"yp{j}_{ot}")
                       for j in range(NCH)]
                for kt in range(KT):
                    for j in range(NCH):
                        nc.tensor.matmul(
                            yps[j][:],
                            weffT_c[:, kt, :],
                            xT_chunks[j][:, kt, :],
                            start=(kt == 0), stop=False,
                        )
                for j in range(NCH):
                    nc.tensor.matmul(
                        yps[j][:],
                        lBT2_sb[:, ot * 128:(ot + 1) * 128],
                        z_sb[:, j * RC:(j + 1) * RC],
                        start=False, stop=True,
                    )
                    y_t = y_pool.tile([128, RC], F32, tag="y")
                    nc.scalar.activation(y_t[:], yps[j][:],
                                         mybir.ActivationFunctionType.Identity,
                                         bias=bias_cols[:, ot:ot + 1],
                                         scale=1.0)
                    nc.gpsimd.dma_start(
                        out=out_ext[ot * 128:(ot + 1) * 128,
                                    j * RC:(j + 1) * RC],
                        in_=y_t[:],
                    )

            NT = O // 128
            weffs = {0: quant(0)}
            if NT > 1:
                weffs[1] = quant(1)
            x_side_and_z()
            for ot in range(NT):
                column(ot, weffs.pop(ot))
                if ot + 2 < NT:
                    weffs[ot + 2] = quant(ot + 2)
    nc.compile()
    return nc


def _shard_inputs(x, weight, scale_A, scale_B, bias, lora_A, lora_B,
                  r_groups=R_GROUPS, o_groups=O_GROUPS):
    rows = x.shape[0]
    outs = weight.shape[0]
    Rs, Os = rows // r_groups, outs // o_groups
    lAT = np.ascontiguousarray(lora_A.T)
    in_maps = []
    for c in range(r_groups * o_groups):
        rg, og = divmod(c, o_groups)
        osl = slice(og * Os, (og + 1) * Os)
        in_maps.append({
            "x": np.ascontiguousarray(x[rg * Rs:(rg + 1) * Rs]),
            "w": np.ascontiguousarray(weight[osl]),
            "sAT": np.ascontiguousarray(scale_A[osl].T),
            "sB": np.ascontiguousarray(scale_B),
            "bias": np.ascontiguousarray(bias[osl][None, :]),
            "lAT": lAT,
            "lBT": np.ascontiguousarray(lora_B[osl].T),
        })
    return in_maps


_NC_CACHE = {}


def kernel(x, weight, scale_A, scale_B, bias, lora_A, lora_B, lut,
           _trace=False):
    from concourse.bass_utils import run_bass_kernel_spmd

    x = np.asarray(x, dtype=np.float32)
    weight = np.asarray(weight, dtype=np.float32)
    scale_A = np.asarray(scale_A, dtype=np.float32)
    scale_B = np.asarray(scale_B, dtype=np.float32)
    bias = np.asarray(bias, dtype=np.float32)
    lora_A = np.asarray(lora_A, dtype=np.float32)
    lora_B = np.asarray(lora_B, dtype=np.float32)
    lut = np.asarray(lut, dtype=np.float32)

    B, S, I = x.shape
    OUT = weight.shape[0]
    xf = x.reshape(B * S, I)
    R = (B * S) // R_GROUPS
    O = OUT // O_GROUPS

    d = np.diff(lut.astype(np.float64))
    affine = np.allclose(d, d[0], rtol=0, atol=1e-6 * max(1.0, np.abs(d[0])))
    lut_a = float(lut[0])
    lut_b = float(d.mean())
    nonaffine = None if affine else lut

    key = (R, O, I, lut_a, lut_b, affine)
    if key not in _NC_CACHE:
        _NC_CACHE[key] = build_nc(R, O, I, lut_a, lut_b,
                                  nonaffine_lut=nonaffine)
    nc = _NC_CACHE[key]

    in_maps = _shard_inputs(xf, weight, scale_A, scale_B, bias, lora_A, lora_B)
    res = run_bass_kernel_spmd(nc, in_maps, core_ids=list(range(N_CORES)),
                               trace=_trace)
    y = np.empty((B * S, OUT), np.float32)
    for c in range(N_CORES):
        rg, og = divmod(c, O_GROUPS)
        y[rg * R:(rg + 1) * R, og * O:(og + 1) * O] = \
            res.results[c]["out"].reshape(O, R).T
    out = y.reshape(B, S, OUT)
    if _trace:
        return out, res
    return out


# revision 24
# speedup vs baseline: 1.4573x; 1.0171x over previous
"""Trainium2 Bass kernel for AnemllQATLinear (fake-quant linear + LoRA + bias).

Math (per reference):
    scales = clip(scale_A @ scale_B, 1e-8)              # [OUT, IN], rank-4
    n      = w / scales
    q      = clip(round((n + 1) / step), 0, 15)         # step = 2/15
    w_q    = lut[q] * scales                            # lut affine: lut[q] = a + b*q
    y      = x @ w_q.T + bias + 2.0 * (x @ lora_A.T) @ lora_B.T

Strategy (8 NeuronCores, 4 row-groups x 2 col-groups):
    Each core gets x rows R=2048 and weight rows (out features) O=2048.
    - Fake-quant computed on-chip arithmetically (affine LUT; round via
      the +/-1.5*2^23 magic trick, which is round-half-even like jnp.round).
    - Quantized weight W_eff converted to bf16, transposed via the DMA
      x-bar into [in, out] layout (DRAM bounce), streamed per o-chunk.
    - x cast f32->bf16 during SWDGE DMA, x-bar transposed to xT resident.
    - Main matmul in bf16: psum[r, o] += xT.T @ weffT, with the LoRA
      term (rank-16) and bias (rank-1) accumulated into the same PSUM
      group as extra matmuls.
    - Output written f32, assembled (concat) on host.
"""

import os
import numpy as np

import concourse.bass as bass
import concourse.tile as tile
from concourse import bacc, mybir

F32 = mybir.dt.float32
F32R = mybir.dt.float32r
BF16 = mybir.dt.bfloat16
MAGIC = 12582912.0  # 1.5 * 2**23
LUT_SIZE = 16
STEP_INV = (LUT_SIZE - 1) / 2.0  # 7.5

# full problem shapes
B_FULL, S_FULL, IN_FULL, OUT_FULL = 4, 2048, 4096, 4096
RANK, LORA_R = 4, 16
R_GROUPS, O_GROUPS = 4, 2
N_CORES = 8


def build_nc(R, O, I, lut_a, lut_b, OC=256, IC=512, nonaffine_lut=None):
    """Build the single-core graph (SPMD-launched on all 8 cores).

    R: x rows per core; O: out features per core; I: contraction dim.
    OC: o-chunk for the main matmul (moving free dim). IC: i-chunk for quant.
    """
    KT = I // 128          # number of 128-wide i (contraction) tiles
    RT = R // 128          # r tiles
    ZC = min(512, R)       # z (lora) accumulation chunk of rows
    assert O % OC == 0 and OC % 128 == 0 and I % IC == 0 and IC % 128 == 0

    nc = bacc.Bacc(None, target_bir_lowering=False, debug=False)

    x_in = nc.declare_dram_parameter("x", [R, I], F32, isOutput=False)
    w_in = nc.declare_dram_parameter("w", [O, I], F32, isOutput=False)
    sAT_in = nc.declare_dram_parameter("sAT", [RANK, O], F32, isOutput=False)
    sB_in = nc.declare_dram_parameter("sB", [RANK, I], F32, isOutput=False)
    bias_in = nc.declare_dram_parameter("bias", [1, O], F32, isOutput=False)
    lAT_in = nc.declare_dram_parameter("lAT", [I, LORA_R], F32, isOutput=False)
    lBT_in = nc.declare_dram_parameter("lBT", [LORA_R, O], F32, isOutput=False)
    out_ext = nc.declare_dram_parameter("out", [O, R], F32, isOutput=True)

    with tile.TileContext(nc) as tc:
        with              tc.tile_pool(name="const", bufs=1) as const_pool, \
             tc.tile_pool(name="xside", bufs=2) as x_pool, \
             tc.tile_pool(name="xT", bufs=1) as xT_pool, \
             tc.tile_pool(name="wload", bufs=2) as w_pool, \
             tc.tile_pool(name="qs", bufs=2) as s_pool, \
             tc.tile_pool(name="qchain", bufs=4) as chain_pool, \
             tc.tile_pool(name="qout", bufs=2) as wq_pool, \
             tc.tile_pool(name="weffc", bufs=3) as weff_pool, \
             tc.tile_pool(name="ysb", bufs=2) as y_pool, \
             tc.tile_pool(name="smallio", bufs=2) as small_pool, \
             tc.tile_pool(name="ps_s", bufs=2, space="PSUM") as psum_s, \
             tc.tile_pool(name="ps_y", bufs=1, space="PSUM") as psum_y, \
             tc.tile_pool(name="ps_z", bufs=2, space="PSUM") as psum_z:

            # ---- constants / small inputs ----
            lAT_sb = const_pool.tile([128, KT, LORA_R], BF16)
            nc.gpsimd.dma_start(
                out=lAT_sb[:],
                in_=lAT_in.rearrange("(kt p) r -> p kt r", p=128),
            )
            lBT2_sb = const_pool.tile([LORA_R, O], BF16)
            nc.gpsimd.dma_start(out=lBT2_sb[:], in_=lBT_in[:, :])  # cast f32->bf16
            bias_cols = const_pool.tile([128, O // 128], F32)
            nc.gpsimd.dma_start(
                out=bias_cols[:],
                in_=bias_in.rearrange("1 (ot p) -> p ot", p=128))
            sB_r = const_pool.tile([RANK, I], F32R)
            for sc in range(I // 256):
                sB_f = small_pool.tile([RANK, 256], F32, tag="sBf")
                nc.gpsimd.dma_start(
                    out=sB_f[:], in_=sB_in[:, sc * 256:(sc + 1) * 256])
                nc.vector.tensor_copy(sB_r[:, sc * 256:(sc + 1) * 256],
                                      sB_f[:])
            z_sb = const_pool.tile([LORA_R, R], BF16)

            # ---- x side: cast to bf16, x-bar transpose into per-chunk xT
            # tiles (finer tiles -> finer scheduler dependencies), with the
            # LoRA z matmuls interleaved per chunk.
            RC = min(512, R)
            NCH = R // RC
            xT_chunks = [xT_pool.tile([128, KT, RC], BF16, name=f"xT_{j}")
                         for j in range(NCH)]
            XH = 4  # quarter-tiles keep the staging pool small

            def x_side():
                for j in range(NCH):
                    for rt in range(j * (RC // 128), (j + 1) * (RC // 128)):
                        for h in range(XH):
                            xbf = x_pool.tile([128, I // XH], BF16, tag="xbf")
                            nc.gpsimd.dma_start(
                                out=xbf[:],
                                in_=x_in[rt * 128:(rt + 1) * 128,
                                         h * (I // XH):(h + 1) * (I // XH)],
                            )
                            lo = (rt % (RC // 128)) * 128
                            nc.sync.dma_start(
                                out=xT_chunks[j][:,
                                                 h * (KT // XH):(h + 1) * (KT // XH),
                                                 lo:lo + 128],
                                in_=xbf[:],
                                transpose=True,
                            )

            def z_chunk(j):
                zp = psum_z.tile([LORA_R, RC], F32, space="PSUM")
                for kt in range(KT):
                    nc.tensor.matmul(
                        zp[:],
                        lAT_sb[:, kt, :],
                        xT_chunks[j][:, kt, :],
                        start=(kt == 0), stop=(kt == KT - 1),
                    )
                # fold the *2.0 LoRA scaling into the psum evacuation
                nc.vector.tensor_scalar(z_sb[:, j * RC:(j + 1) * RC], zp[:],
                                        2.0, None, op0=mybir.AluOpType.mult)

            # ---- main loop: software-pipelined by one o-tile: quant(ot+1) is
            # traced BEFORE the matmuls of ot so every engine queue (PE's
            # scales-mm especially) runs the next tile's quant work while the
            # current column's matmuls stream.

            def quant(ot):
                """Quantize weight o-tile `ot` into a fresh weffT tile."""
                weffT_c = weff_pool.tile([128, KT, 128], BF16, tag="weff",
                                         name=f"weff_{ot}")
                sAT_f = small_pool.tile([RANK, 128], F32, tag="sATf")
                nc.scalar.dma_start(
                    out=sAT_f[:], in_=sAT_in[:, ot * 128:(ot + 1) * 128])
                sAT_t = small_pool.tile([RANK, 128], F32R, tag="sAT")
                nc.vector.tensor_copy(sAT_t[:], sAT_f[:])
                for ic in range(I // IC):
                    w_t = w_pool.tile([128, IC], F32, tag="w")
                    nc.scalar.dma_start(
                        out=w_t[:],
                        in_=w_in[ot * 128:(ot + 1) * 128, ic * IC:(ic + 1) * IC],
                    )
                    sp = psum_s.tile([128, IC], F32, space="PSUM")
                    nc.tensor.matmul(sp[:], sAT_t[:],
                                     sB_r[:, ic * IC:(ic + 1) * IC],
                                     start=True, stop=True)
                    s_t = s_pool.tile([128, IC], F32, tag="s")
                    nc.vector.tensor_scalar(s_t[:], sp[:], 1e-8, None,
                                            op0=mybir.AluOpType.max)
                    r_t = chain_pool.tile([128, IC], F32, tag="chain")
                    nc.vector.reciprocal_approx_fast(r_t[:], s_t[:])
                    n_t = chain_pool.tile([128, IC], F32, tag="chain")
                    nc.vector.tensor_tensor(n_t[:], w_t[:], r_t[:],
                                            op=mybir.AluOpType.mult)
                    v_t = chain_pool.tile([128, IC], F32, tag="chain")
                    nc.vector.tensor_scalar(v_t[:], n_t[:], STEP_INV, -0.5,
                                            op0=mybir.AluOpType.mult,
                                            op1=mybir.AluOpType.add)
                    t_t = chain_pool.tile([128, IC], F32, tag="chain")
                    nc.vector.tensor_scalar(t_t[:], v_t[:], MAGIC + 8.0, MAGIC,
                                            op0=mybir.AluOpType.add,
                                            op1=mybir.AluOpType.subtract)
                    q_t = chain_pool.tile([128, IC], F32, tag="chain")
                    nc.vector.tensor_scalar(q_t[:], t_t[:],
                                            float(LUT_SIZE - 1), 0.0,
                                            op0=mybir.AluOpType.min,
                                            op1=mybir.AluOpType.max)
                    u_t = chain_pool.tile([128, IC], F32, tag="chain")
                    if nonaffine_lut is None:
                        nc.vector.tensor_scalar(u_t[:], q_t[:], float(lut_b),
                                                float(lut_a),
                                                op0=mybir.AluOpType.mult,
                                                op1=mybir.AluOpType.add)
                    else:
                        # generic LUT: u = lut[0] + sum_k d_k*(q >= k-0.5)
                        lut = nonaffine_lut
                        acc = chain_pool.tile([128, IC], F32, tag="chain")
                        nc.vector.tensor_scalar(acc[:], q_t[:], 0.0, float(lut[0]),
                                                op0=mybir.AluOpType.mult,
                                                op1=mybir.AluOpType.add)
                        for k in range(1, LUT_SIZE):
                            d_k = float(lut[k] - lut[k - 1])
                            ind = chain_pool.tile([128, IC], F32, tag="chain")
                            nc.vector.tensor_scalar(ind[:], q_t[:], k - 0.5, d_k,
                                                    op0=mybir.AluOpType.is_ge,
                                                    op1=mybir.AluOpType.mult)
                            acc2 = chain_pool.tile([128, IC], F32, tag="chain")
                            nc.vector.tensor_tensor(acc2[:], acc[:], ind[:],
                                                    op=mybir.AluOpType.add)
                            acc = acc2
                        u_t = acc
                    wq_t = wq_pool.tile([128, IC], BF16, tag="wq")
                    nc.vector.tensor_tensor(wq_t[:], u_t[:], s_t[:],
                                            op=mybir.AluOpType.mult)
                    # transpose straight into the weffT tile
                    nc.sync.dma_start(
                        out=weffT_c[:, ic * (IC // 128):(ic + 1) * (IC // 128), :],
                        in_=wq_t[:],
                        transpose=True,
                    )
                return weffT_c

            def column(ot, weffT_c, with_z=False):
                """yT[o_tile, :] = W_eff[o_tile] @ x.T + lora; bias folds
                into the ACT psum evacuation as a per-partition offset."""
                yps = [psum_y.tile([128, RC], F32, space="PSUM",
                                   tag=f"yp{j}", name=f"yp{j}_{ot}")
                       for j in range(NCH)]
                for kt in range(KT):
                    for j in range(NCH):
                        nc.tensor.matmul(
                            yps[j][:],
                            weffT_c[:, kt, :],
                            xT_chunks[j][:, kt, :],
                            start=(kt == 0), stop=False,
                        )
                if with_z:
                    for j in range(NCH):
                        z_chunk(j)
                for j in range(NCH):
                    nc.tensor.matmul(
                        yps[j][:],
                        lBT2_sb[:, ot * 128:(ot + 1) * 128],
                        z_sb[:, j * RC:(j + 1) * RC],
                        start=False, stop=True,
                    )
                    y_t = y_pool.tile([128, RC], F32, tag="y")
                    nc.scalar.activation(y_t[:], yps[j][:],
                                         mybir.ActivationFunctionType.Identity,
                                         bias=bias_cols[:, ot:ot + 1],
                                         scale=1.0)
                    nc.gpsimd.dma_start(
                        out=out_ext[ot * 128:(ot + 1) * 128,
                                    j * RC:(j + 1) * RC],
                        in_=y_t[:],
                    )

            NT = O // 128
            weffs = {0: quant(0)}
            if NT > 1:
                weffs[1] = quant(1)
            x_side()
            for ot in range(NT):
                column(ot, weffs.pop(ot), with_z=(ot == 0))
                if ot + 2 < NT:
                    weffs[ot + 2] = quant(ot + 2)
    nc.compile()
    return nc


def _shard_inputs(x, weight, scale_A, scale_B, bias, lora_A, lora_B,
                  r_groups=R_GROUPS, o_groups=O_GROUPS):
    rows = x.shape[0]
    outs = weight.shape[0]
    Rs, Os = rows // r_groups, outs // o_groups
    lAT = np.ascontiguousarray(lora_A.T)
    in_maps = []
    for c in range(r_groups * o_groups):
        rg, og = divmod(c, o_groups)
        osl = slice(og * Os, (og + 1) * Os)
        in_maps.append({
            "x": np.ascontiguousarray(x[rg * Rs:(rg + 1) * Rs]),
            "w": np.ascontiguousarray(weight[osl]),
            "sAT": np.ascontiguousarray(scale_A[osl].T),
            "sB": np.ascontiguousarray(scale_B),
            "bias": np.ascontiguousarray(bias[osl][None, :]),
            "lAT": lAT,
            "lBT": np.ascontiguousarray(lora_B[osl].T),
        })
    return in_maps


_NC_CACHE = {}


def kernel(x, weight, scale_A, scale_B, bias, lora_A, lora_B, lut,
           _trace=False):
    from concourse.bass_utils import run_bass_kernel_spmd

    x = np.asarray(x, dtype=np.float32)
    weight = np.asarray(weight, dtype=np.float32)
    scale_A = np.asarray(scale_A, dtype=np.float32)
    scale_B = np.asarray(scale_B, dtype=np.float32)
    bias = np.asarray(bias, dtype=np.float32)
    lora_A = np.asarray(lora_A, dtype=np.float32)
    lora_B = np.asarray(lora_B, dtype=np.float32)
    lut = np.asarray(lut, dtype=np.float32)

    B, S, I = x.shape
    OUT = weight.shape[0]
    xf = x.reshape(B * S, I)
    R = (B * S) // R_GROUPS
    O = OUT // O_GROUPS

    d = np.diff(lut.astype(np.float64))
    affine = np.allclose(d, d[0], rtol=0, atol=1e-6 * max(1.0, np.abs(d[0])))
    lut_a = float(lut[0])
    lut_b = float(d.mean())
    nonaffine = None if affine else lut

    key = (R, O, I, lut_a, lut_b, affine)
    if key not in _NC_CACHE:
        _NC_CACHE[key] = build_nc(R, O, I, lut_a, lut_b,
                                  nonaffine_lut=nonaffine)
    nc = _NC_CACHE[key]

    in_maps = _shard_inputs(xf, weight, scale_A, scale_B, bias, lora_A, lora_B)
    res = run_bass_kernel_spmd(nc, in_maps, core_ids=list(range(N_CORES)),
                               trace=_trace)
    y = np.empty((B * S, OUT), np.float32)
    for c in range(N_CORES):
        rg, og = divmod(c, O_GROUPS)
        y[rg * R:(rg + 1) * R, og * O:(og + 1) * O] = \
            res.results[c]["out"].reshape(O, R).T
    out = y.reshape(B, S, OUT)
    if _trace:
        return out, res
    return out
